# revision 25
# baseline (speedup 1.0000x reference)
"""GNN message-passing kernel for 8 Trainium2 NeuronCores (Bass/Tile).

Takes FULL inputs, shards nodes across 8 cores internally, runs the
4-layer GNN (dense -> spmm -> spmm -> dense) with two bf16 AllGathers
of the hidden node table (each split in two chunks, fired as soon as
the producing blocks finish), and PE-matmul-based weighted segment
sums.  The one-hot selector matrices are built ON-CHIP on the vector
engine from per-edge (local-row, weight) pairs; the row gathers are
spread round-robin over 4 SWDGE queues so descriptor generation runs
on all four GpSimd Q7 core-pairs in parallel.
"""

import math
from contextlib import ExitStack
from dataclasses import dataclass

import ml_dtypes
import numpy as np

import concourse.bass as bass
import concourse.mybir as mybir
import concourse.tile as tile
from concourse import bacc
from concourse.bass_utils import run_bass_kernel_spmd
from concourse.masks import make_identity

BF16 = ml_dtypes.bfloat16
AF = mybir.ActivationFunctionType
ALU = mybir.AluOpType


@dataclass(frozen=True)
class Cfg:
    n_nodes: int = 50000
    n_edges: int = 800000
    in_dim: int = 512
    h1: int = 512
    h2: int = 256
    out_dim: int = 128
    n_cores: int = 8
    split_block: int = 28  # blocks [0, 28) -> half A, [28, 49) -> half B
    slab_blocks: int = 4   # L1/L2a node-slab width in 128-blocks

    @property
    def nodes_per_core(self):
        return self.n_nodes // self.n_cores  # 6250

    @property
    def npad(self):
        return math.ceil(self.nodes_per_core / 128) * 128  # 6272

    @property
    def nblocks(self):
        return self.npad // 128  # 49

    @property
    def rows_a(self):
        return self.split_block * 128  # 3584

    @property
    def rows_b(self):
        return self.npad - self.rows_a  # 2688

    @property
    def tab_a(self):
        return self.rows_a * self.n_cores  # 28672

    @property
    def tab_b(self):
        return self.rows_b * self.n_cores  # 21504


FULL = Cfg()


# ---------------------------------------------------------------- host prep


def edge_structure(cfg: Cfg, edge_row, edge_col, edge_weight):
    """Bucket edges per (core, row-block, table-half); uniform chunk counts.

    Returns (meta, per_core):
      meta['nch'][b][h]   chunks for block b, half h (same on all cores)
      meta['off16'][b][h] idx-tile int16-column offset of that bucket
      meta['offch'][b][h] chunk offset (for the lrw tile)
      meta['totch']       total chunks
      meta['idxcols']     int16 columns of the idx tensor
    per_core[c] = dict(idx=[128, idxcols] int16, lrw=[128, totch*2] bf16)
    """
    nc_, npad, nb = cfg.n_cores, cfg.npad, cfg.nblocks
    npc = cfg.nodes_per_core
    rows_a = cfg.rows_a
    assert cfg.tab_a <= 32767 and cfg.tab_b <= 32767

    core_of = edge_row // npc
    lr_all = edge_row - core_of * npc          # local dest row on owner core
    cc_of = edge_col // npc                    # core owning the source col
    cl_all = edge_col - cc_of * npc            # local source row
    half_all = (cl_all >= rows_a).astype(np.int64)
    # index within the half table
    tidx_all = np.where(
        half_all == 0,
        cc_of * rows_a + cl_all,
        cc_of * cfg.rows_b + (cl_all - rows_a),
    )

    counts = np.zeros((nc_, nb, 2), np.int64)
    per = {}
    for c in range(nc_):
        m = core_of == c
        lr, ti, hf, w = lr_all[m], tidx_all[m], half_all[m], edge_weight[m]
        blk = lr // 128
        order = np.lexsort((ti, hf, blk))  # sort by (block, half, table idx)
        per[c] = (lr[order], ti[order], hf[order], w[order], blk[order])
        np.add.at(counts[c], (blk, hf), 1)

    chunks_bh = np.ceil(counts / 128.0).astype(np.int64).max(axis=0)  # [nb,2]
    chunks_bh = np.maximum(chunks_bh, 1)

    nch = [[int(chunks_bh[b, h]) for h in (0, 1)] for b in range(nb)]
    off16 = [[0, 0] for _ in range(nb)]
    offch = [[0, 0] for _ in range(nb)]
    tot16 = 0
    totch = 0
    for b in range(nb):
        for h in (0, 1):
            off16[b][h] = tot16
            offch[b][h] = totch
            tot16 += nch[b][h] * 8  # 128 idx per chunk -> 8 int16 cols
            totch += nch[b][h]

    meta = dict(nch=nch, off16=off16, offch=offch, totch=totch, idxcols=tot16)

    per_core = []
    for c in range(nc_):
        lr, ti, hf, w, blk = per[c]
        idx_flat = np.zeros(tot16 * 16, np.int16)
        lr_tab = np.zeros((128, totch), BF16)
        w_tab = np.zeros((128, totch), BF16)
        bucket_no = 0
        p = 0  # cursor into sorted edge stream
        for b in range(nb):
            for h in (0, 1):
                q = p
                while q < len(blk) and blk[q] == b and hf[q] == h:
                    q += 1
                e_ti, e_lr, e_w = ti[p:q], lr[p:q], w[p:q]
                p = q
                n = len(e_ti)
                nslots = nch[b][h] * 128
                pad = nslots - n
                # trailing pads: -1 (ucode trims them) except the first few
                # buckets, whose gather tiles read never-written SBUF.
                pad_idx = 0
                bucket_no += 1
                ti_pad = np.concatenate(
                    [e_ti, np.full(pad, pad_idx, np.int64)]
                )
                i_in = np.arange(nslots)
                base16 = off16[b][h]
                idx_flat[(base16 + i_in // 16) * 16 + (i_in % 16)] = ti_pad.astype(
                    np.int16
                )
                if n > 0:
                    j0 = offch[b][h]
                    i_e = np.arange(n)
                    jj = j0 + i_e // 128
                    ss = i_e % 128
                    lr_tab[ss, jj] = (e_lr - b * 128).astype(BF16)
                    w_tab[ss, jj] = e_w.astype(BF16)
        idx_mat = idx_flat.reshape(tot16, 16).T  # [16, idxcols]
        idx_mat = np.tile(idx_mat, (8, 1))       # replicate to 128 partitions
        per_core.append(
            dict(
                idx=np.ascontiguousarray(idx_mat),
                lrt=np.ascontiguousarray(lr_tab),
                wt=np.ascontiguousarray(w_tab),
            )
        )
    return meta, per_core


def prep_inputs(cfg: Cfg, inputs):
    f = inputs["features"].astype(np.float32)
    meta, per_edge = edge_structure(
        cfg,
        inputs["edge_row"].astype(np.int64),
        inputs["edge_col"].astype(np.int64),
        inputs["edge_weight"].astype(np.float32),
    )
    kin = cfg.in_dim // 128
    k1 = cfg.h1 // 128
    k2 = cfg.h2 // 128

    def wlayout(w, kt):
        K, M = w.shape
        return (
            w.reshape(kt, 128, M).transpose(1, 0, 2).reshape(128, kt * M)
        ).astype(BF16)

    w1 = wlayout(inputs["W_lin1"].astype(np.float32), kin)
    wg1 = wlayout(inputs["W_g1"].astype(np.float32), k1)
    wg2 = wlayout(inputs["W_g2"].astype(np.float32), k2)
    wl2 = wlayout(inputs["W_lin2"].astype(np.float32), k2)
    b1 = inputs["b_lin1"].astype(np.float32).reshape(kin, 128).T.copy()
    bg1 = inputs["b_g1"].astype(BF16).reshape(1, cfg.h2)
    bg2 = inputs["b_g2"].astype(BF16).reshape(1, cfg.h2)
    bl2 = inputs["b_lin2"].astype(BF16).reshape(1, cfg.out_dim)

    npc, npad = cfg.nodes_per_core, cfg.npad
    in_maps = []
    for c in range(cfg.n_cores):
        lo = c * npc
        hi = min((c + 1) * npc, cfg.n_nodes)
        xc = np.zeros((npad, cfg.in_dim), np.float32)
        xc[: hi - lo] = f[lo:hi]
        xt = (
            xc.T.reshape(kin, 128, npad)
            .transpose(1, 0, 2)
            .reshape(128, kin * npad)
        ).astype(BF16)
        in_maps.append(
            {
                "xt": np.ascontiguousarray(xt),
                "w1": w1,
                "wg1": wg1,
                "wg2": wg2,
                "wl2": wl2,
                "b1": b1,
                "bg1": bg1,
                "bg2": bg2,
                "bl2": bl2,
                "idx": per_edge[c]["idx"],
                "lrt": per_edge[c]["lrt"],
                "wt": per_edge[c]["wt"],
            }
        )
    return meta, in_maps


# ---------------------------------------------------------------- kernel IR


def build(cfg: Cfg, meta):
    nc = bacc.Bacc(
        "TRN2",
        target_bir_lowering=False,
        debug=False,
        num_devices=cfg.n_cores,
        num_swdge_queues=4,
    )
    bf = mybir.dt.bfloat16
    f32 = mybir.dt.float32
    i16 = mybir.dt.int16
    kin = cfg.in_dim // 128
    k1 = cfg.h1 // 128
    k2 = cfg.h2 // 128
    npad, nb, H2, OUT = cfg.npad, cfg.nblocks, cfg.h2, cfg.out_dim
    SB = cfg.split_block
    totch = meta["totch"]
    nch = meta["nch"]
    off16 = meta["off16"]
    offch = meta["offch"]

    xt_d = nc.dram_tensor("xt", [128, kin * npad], bf, kind="ExternalInput").ap()
    w1_d = nc.dram_tensor("w1", [128, kin * cfg.h1], bf, kind="ExternalInput").ap()
    wg1_d = nc.dram_tensor("wg1", [128, k1 * H2], bf, kind="ExternalInput").ap()
    wg2_d = nc.dram_tensor("wg2", [128, k2 * H2], bf, kind="ExternalInput").ap()
    wl2_d = nc.dram_tensor("wl2", [128, k2 * OUT], bf, kind="ExternalInput").ap()
    b1_d = nc.dram_tensor("b1", [128, kin], f32, kind="ExternalInput").ap()
    bg1_d = nc.dram_tensor("bg1", [1, H2], bf, kind="ExternalInput").ap()
    bg2_d = nc.dram_tensor("bg2", [1, H2], bf, kind="ExternalInput").ap()
    bl2_d = nc.dram_tensor("bl2", [1, OUT], bf, kind="ExternalInput").ap()
    idx_d = nc.dram_tensor(
        "idx", [128, meta["idxcols"]], i16, kind="ExternalInput"
    ).ap()
    lrt_d = nc.dram_tensor("lrt", [128, totch], bf, kind="ExternalInput").ap()
    wt_d = nc.dram_tensor("wt", [128, totch], bf, kind="ExternalInput").ap()
    y_d = nc.dram_tensor("y", [npad, OUT], f32, kind="ExternalOutput").ap()

    g1locA = nc.dram_tensor("g1locA", [cfg.rows_a, H2], bf).ap()
    g1locB = nc.dram_tensor("g1locB", [cfg.rows_b, H2], bf).ap()
    g2locA = nc.dram_tensor("g2locA", [cfg.rows_a, H2], bf).ap()
    g2locB = nc.dram_tensor("g2locB", [cfg.rows_b, H2], bf).ap()
    g1tabA = nc.dram_tensor("g1tabA", [cfg.tab_a, H2], bf, addr_space="Shared").ap()
    g1tabB = nc.dram_tensor("g1tabB", [cfg.tab_b, H2], bf, addr_space="Shared").ap()
    g2tabA = nc.dram_tensor("g2tabA", [cfg.tab_a, H2], bf, addr_space="Shared").ap()
    g2tabB = nc.dram_tensor("g2tabB", [cfg.tab_b, H2], bf, addr_space="Shared").ap()

    rg = [list(range(cfg.n_cores))]
    qctr = [0]  # round-robin SWDGE queue counter

    def fire_ag(src, dst):
        nc.gpsimd.collective_compute(
            "AllGather",
            mybir.AluOpType.bypass,
            replica_groups=rg,
            ins=[src[:, :]],
            outs=[dst[:, :]],
        )

    with tile.TileContext(nc) as tc:
        with ExitStack() as top:
            const = top.enter_context(tc.tile_pool(name="const", bufs=1))
            w1_s = const.tile([128, kin * cfg.h1], bf)
            nc.sync.dma_start(w1_s[:], w1_d[:, :])
            wg1_s = const.tile([128, k1 * H2], bf)
            nc.sync.dma_start(wg1_s[:], wg1_d[:, :])
            wg2_s = const.tile([128, k2 * H2], bf)
            nc.sync.dma_start(wg2_s[:], wg2_d[:, :])
            wl2_s = const.tile([128, k2 * OUT], bf)
            nc.sync.dma_start(wl2_s[:], wl2_d[:, :])
            b1_s = const.tile([128, kin], f32)
            nc.sync.dma_start(b1_s[:], b1_d[:, :])
            bg1_s = const.tile([1, H2], bf)
            nc.sync.dma_start(bg1_s[:], bg1_d[:, :])
            bg2_s = const.tile([1, H2], bf)
            nc.sync.dma_start(bg2_s[:], bg2_d[:, :])
            bl2_s = const.tile([1, OUT], bf)
            nc.sync.dma_start(bl2_s[:], bl2_d[:, :])
            idx_s = const.tile([128, meta["idxcols"]], i16)
            nc.sync.dma_start(idx_s[:], idx_d[:, :])
            lrt_s = const.tile([128, totch], bf)
            nc.sync.dma_start(lrt_s[:], lrt_d[:, :])
            wt_s = const.tile([128, totch], bf)
            nc.sync.dma_start(wt_s[:], wt_d[:, :])
            iota_i = const.tile([128, 128], i16)
            nc.gpsimd.iota(iota_i[:], pattern=[[1, 128]], base=0, channel_multiplier=0)
            iota_b = const.tile([128, 128], bf)
            nc.vector.tensor_copy(iota_b[:], iota_i[:])
            ident = const.tile([128, 128], bf)
            make_identity(nc, ident[:])
            ones_t = const.tile([1, 128], bf)
            nc.gpsimd.memset(ones_t[:], 1.0)

            # ---------------- L1 + L2a, slab-streamed; fire AG1 chunks early
            slabs = []
            b0 = 0
            while b0 < nb:
                b1e = min(b0 + cfg.slab_blocks, nb)
                slabs.append((b0, b1e))
                b0 = b1e

            with ExitStack() as pl1:
                xp = pl1.enter_context(tc.tile_pool(name="xt", bufs=2))
                hp = pl1.enter_context(tc.tile_pool(name="h1s", bufs=2))
                ps1 = pl1.enter_context(
                    tc.tile_pool(name="ps1", bufs=4, space="PSUM")
                )
                ps2 = pl1.enter_context(
                    tc.tile_pool(name="ps2", bufs=2, space="PSUM")
                )
                gp1 = pl1.enter_context(tc.tile_pool(name="g1t", bufs=3))
                for (bs, be) in slabs:
                    a = bs * 128
                    S = (be - bs) * 128
                    xs = xp.tile([128, kin, S], bf, tag="x")
                    for kt in range(kin):
                        nc.sync.dma_start(
                            xs[:, kt, :], xt_d[:, kt * npad + a : kt * npad + a + S]
                        )
                    h1s = hp.tile([128, k1, S], bf, tag="h")
                    for f1t in range(k1):
                        ps = ps1.tile([128, S], f32, tag="ps")
                        for kt in range(kin):
                            nc.tensor.matmul(
                                ps[:],
                                lhsT=w1_s[
                                    :,
                                    kt * cfg.h1 + f1t * 128 : kt * cfg.h1
                                    + f1t * 128
                                    + 128,
                                ],
                                rhs=xs[:, kt, :],
                                start=(kt == 0),
                                stop=(kt == kin - 1),
                            )
                        nc.scalar.activation(
                            h1s[:, f1t, :],
                            ps[:],
                            AF.Sigmoid,
                            bias=b1_s[:, f1t : f1t + 1],
                        )
                    for b in range(bs, be):
                        o = (b - bs) * 128
                        ps = ps2.tile([128, H2], f32, tag="ps")
                        for kt in range(k1):
                            nc.tensor.matmul(
                                ps[:],
                                lhsT=h1s[:, kt, o : o + 128],
                                rhs=wg1_s[:, kt * H2 : (kt + 1) * H2],
                                start=(kt == 0),
                                stop=(kt == k1 - 1),
                            )
                        g1t = gp1.tile([128, H2], bf, tag="g1")
                        nc.vector.tensor_copy(g1t[:], ps[:])
                        if b < SB:
                            nc.sync.dma_start(
                                g1locA[b * 128 : (b + 1) * 128, :], g1t[:]
                            )
                        else:
                            bb = b - SB
                            nc.sync.dma_start(
                                g1locB[bb * 128 : (bb + 1) * 128, :], g1t[:]
                            )
                    if be == SB:
                        fire_ag(g1locA, g1tabA)
                assert SB in [be for (_, be) in slabs], "split must be slab-aligned"
                fire_ag(g1locB, g1tabB)

            # Dependency-free PE filler: keeps the HAM activity window hot
            # across phase transitions where the PE would otherwise idle
            # waiting for an AllGather to land (idle >3.4us drops the PE
            # clock from 2.4 to 1.2 GHz for the next stretch of work).
            fillp = top.enter_context(
                tc.tile_pool(name="fill", bufs=1, space="PSUM")
            )

            def pe_filler(n_mm, tag):
                fps = fillp.tile([128, 256], f32, tag="f", name=f"f{tag}")
                for i in range(n_mm):
                    nc.tensor.matmul(
                        fps[:],
                        lhsT=w1_s[:, :128],
                        rhs=w1_s[:, :256],
                        start=True,
                        stop=True,
                    )

            # ---------------- shared spmm machinery (pools shared across
            # both layers so recycled gather tiles always hold prior gather
            # data -- required for the trailing -1 idx-pad trim)
            nch_max = max(max(r) for r in nch)
            nctot_max = max(r[0] + r[1] for r in nch)
            gp = top.enter_context(tc.tile_pool(name="gath", bufs=10))
            pp = top.enter_context(tc.tile_pool(name="pm", bufs=3))
            sp = top.enter_context(tc.tile_pool(name="psmm", bufs=4, space="PSUM"))

            def spmm_layer(ctx, tabA, tabB, brow, out_cb, tag):
                # process blocks in pairs: one batched P-build per pair
                # (chunk ids are block-major contiguous, so a pair's chunks
                # [offch[b0][0], offch[b0][0]+nct2) are one contiguous range)
                for b0p in range(0, nb, 2):
                    pair = [b for b in (b0p, b0p + 1) if b < nb]
                    tiles = {}
                    for b in pair:
                        for h in (0, 1):
                            n = nch[b][h]
                            t = gp.tile([128, nch_max, H2], bf, tag="g")
                            tab = tabA if h == 0 else tabB
                            for lo in range(0, n, 15):
                                ns = min(15, n - lo)
                                o16 = off16[b][h] + lo * 8
                                nc.gpsimd.dma_gather(
                                    out_ap=t[:, lo : lo + ns, :],
                                    in_ap=tab[:, :],
                                    idxs_ap=idx_s[:, o16 : o16 + ns * 8],
                                    num_idxs=ns * 128,
                                    num_idxs_reg=ns * 128,
                                    elem_size=H2,
                                    single_packet=False,
                                    queue_num=qctr[0] % 4,
                                )
                                qctr[0] += 1
                            tiles[(b, h)] = t
                    j0 = offch[pair[0]][0]
                    nct2 = sum(nch[b][h] for b in pair for h in (0, 1))
                    pt = pp.tile([128, 2 * nctot_max, 128], bf, tag="p")
                    nc.vector.tensor_tensor(
                        pt[:, :nct2, :],
                        iota_b[:].unsqueeze(1).broadcast_to((128, nct2, 128)),
                        lrt_s[:, j0 : j0 + nct2]
                        .unsqueeze(2)
                        .broadcast_to((128, nct2, 128)),
                        ALU.is_equal,
                    )
                    nc.vector.tensor_tensor(
                        pt[:, :nct2, :],
                        pt[:, :nct2, :],
                        wt_s[:, j0 : j0 + nct2]
                        .unsqueeze(2)
                        .broadcast_to((128, nct2, 128)),
                        ALU.mult,
                    )
                    for b in pair:
                        ps = sp.tile(
                            [128, H2], f32, tag="ps", name=f"psmm{tag}_{b}"
                        )
                        first = True
                        for h in (0, 1):
                            for j in range(nch[b][h]):
                                jl = (offch[b][h] - j0) + j
                                nc.tensor.matmul(
                                    ps[:],
                                    lhsT=pt[:, jl, :],
                                    rhs=tiles[(b, h)][:, j, :],
                                    start=first,
                                    stop=False,
                                )
                                first = False
                        nc.tensor.matmul(
                            ps[:],
                            lhsT=ones_t[:1, :],
                            rhs=brow[:1, :],
                            start=first,
                            stop=True,
                        )
                        out_cb(b, ps)

            # ---------------- spmm1 + L3a fused per block; fire AG2 chunks
            with ExitStack() as s1:
                tps3 = s1.enter_context(
                    tc.tile_pool(name="tps3", bufs=1, space="PSUM")
                )
                psp3 = s1.enter_context(
                    tc.tile_pool(name="ps3", bufs=2, space="PSUM")
                )
                tp3 = s1.enter_context(tc.tile_pool(name="l3t", bufs=3))

                def cb1(b, psum):
                    h2t = tp3.tile([128, H2], bf, tag="h2")
                    nc.scalar.activation(h2t[:], psum[:], AF.Relu)
                    h2T = tp3.tile([128, k2, 128], bf, tag="h2T")
                    for kt in range(k2):
                        ptt = tps3.tile([128, 128], bf, tag="pt")
                        nc.tensor.transpose(
                            ptt[:], h2t[:, kt * 128 : (kt + 1) * 128], ident[:]
                        )
                        nc.scalar.activation(h2T[:, kt, :], ptt[:], AF.Copy)
                    ps3 = psp3.tile([128, H2], f32, tag="ps")
                    for kt in range(k2):
                        nc.tensor.matmul(
                            ps3[:],
                            lhsT=h2T[:, kt, :],
                            rhs=wg2_s[:, kt * H2 : (kt + 1) * H2],
                            start=(kt == 0),
                            stop=(kt == k2 - 1),
                        )
                    g2t = tp3.tile([128, H2], bf, tag="g2")
                    nc.scalar.activation(g2t[:], ps3[:], AF.Copy)
                    if b < SB:
                        nc.sync.dma_start(g2locA[b * 128 : (b + 1) * 128, :], g2t[:])
                        if b == SB - 1:
                            fire_ag(g2locA, g2tabA)
                    else:
                        bb = b - SB
                        nc.sync.dma_start(
                            g2locB[bb * 128 : (bb + 1) * 128, :], g2t[:]
                        )
                        if b == nb - 1:
                            fire_ag(g2locB, g2tabB)

                pe_filler(150, "a")
                spmm_layer(s1, g1tabA, g1tabB, bg1_s, cb1, "a")

            # ---------------- spmm2 + L4 fused per block
            with ExitStack() as s2:
                tps4 = s2.enter_context(
                    tc.tile_pool(name="tps4", bufs=1, space="PSUM")
                )
                psp4 = s2.enter_context(
                    tc.tile_pool(name="ps4", bufs=2, space="PSUM")
                )
                tp4 = s2.enter_context(tc.tile_pool(name="l4t", bufs=3))

                def cb2(b, psum):
                    h3t = tp4.tile([128, H2], bf, tag="h3")
                    nc.scalar.activation(h3t[:], psum[:], AF.Relu)
                    h3T = tp4.tile([128, k2, 128], bf, tag="h3T")
                    for kt in range(k2):
                        ptt = tps4.tile([128, 128], bf, tag="pt")
                        nc.tensor.transpose(
                            ptt[:], h3t[:, kt * 128 : (kt + 1) * 128], ident[:]
                        )
                        nc.scalar.activation(h3T[:, kt, :], ptt[:], AF.Copy)
                    ps4 = psp4.tile([128, OUT], f32, tag="ps")
                    for kt in range(k2):
                        nc.tensor.matmul(
                            ps4[:],
                            lhsT=h3T[:, kt, :],
                            rhs=wl2_s[:, kt * OUT : (kt + 1) * OUT],
                            start=(kt == 0),
                            stop=False,
                        )
                    nc.tensor.matmul(
                        ps4[:],
                        lhsT=ones_t[:1, :],
                        rhs=bl2_s[:1, :],
                        start=False,
                        stop=True,
                    )
                    yt = tp4.tile([128, OUT], f32, tag="y")
                    nc.scalar.activation(yt[:], ps4[:], AF.Copy)
                    nc.sync.dma_start(y_d[b * 128 : (b + 1) * 128, :], yt[:])

                pe_filler(100, "b")
                spmm_layer(s2, g2tabA, g2tabB, bg2_s, cb2, "b")

    nc.compile()
    return nc


# ---------------------------------------------------------------- driver

_CACHE = {}


def run(inputs, cfg: Cfg = FULL, trace=False, tmpdir=None):
    meta, in_maps = prep_inputs(cfg, inputs)
    key = (cfg, meta["totch"], meta["idxcols"])
    if key not in _CACHE:
        _CACHE[key] = build(cfg, meta)
    nc = _CACHE[key]
    res = run_bass_kernel_spmd(
        nc,
        in_maps,
        core_ids=list(range(cfg.n_cores)),
        trace=trace,
        tmpdir=tmpdir,
    )
    npc = cfg.nodes_per_core
    out = np.empty((cfg.n_nodes, cfg.out_dim), np.float32)
    for c in range(cfg.n_cores):
        lo = c * npc
        hi = min((c + 1) * npc, cfg.n_nodes)
        out[lo:hi] = res.results[c]["y"][: hi - lo]
    return out, res


def kernel(**inputs) -> np.ndarray:
    out, _ = run(inputs, FULL, trace=False)
    return out


# revision 27
# speedup vs baseline: 1.0151x; 1.0151x over previous
"""GNN message-passing kernel for 8 Trainium2 NeuronCores (Bass/Tile).

Takes FULL inputs, shards nodes across 8 cores internally, runs the
4-layer GNN (dense -> spmm -> spmm -> dense) with two bf16 AllGathers
of the hidden node table (each split in two chunks, fired as soon as
the producing blocks finish), and PE-matmul-based weighted segment
sums.  The one-hot selector matrices are built ON-CHIP on the vector
engine from per-edge (local-row, weight) pairs; the row gathers are
spread round-robin over 4 SWDGE queues so descriptor generation runs
on all four GpSimd Q7 core-pairs in parallel.
"""

import math
from contextlib import ExitStack
from dataclasses import dataclass

import ml_dtypes
import numpy as np

import concourse.bass as bass
import concourse.mybir as mybir
import concourse.tile as tile
from concourse import bacc
from concourse.bass_utils import run_bass_kernel_spmd
from concourse.masks import make_identity

BF16 = ml_dtypes.bfloat16
AF = mybir.ActivationFunctionType
ALU = mybir.AluOpType


@dataclass(frozen=True)
class Cfg:
    n_nodes: int = 50000
    n_edges: int = 800000
    in_dim: int = 512
    h1: int = 512
    h2: int = 256
    out_dim: int = 128
    n_cores: int = 8
    split_block: int = 28  # blocks [0, 28) -> half A, [28, 49) -> half B
    slab_blocks: int = 4   # L1/L2a node-slab width in 128-blocks

    @property
    def nodes_per_core(self):
        return self.n_nodes // self.n_cores  # 6250

    @property
    def npad(self):
        return math.ceil(self.nodes_per_core / 128) * 128  # 6272

    @property
    def nblocks(self):
        return self.npad // 128  # 49

    @property
    def rows_a(self):
        return self.split_block * 128  # 3584

    @property
    def rows_b(self):
        return self.npad - self.rows_a  # 2688

    @property
    def tab_a(self):
        return self.rows_a * self.n_cores  # 28672

    @property
    def tab_b(self):
        return self.rows_b * self.n_cores  # 21504


FULL = Cfg()


# ---------------------------------------------------------------- host prep


def edge_structure(cfg: Cfg, edge_row, edge_col, edge_weight):
    """Bucket edges per (core, row-block, table-half); uniform chunk counts.

    Returns (meta, per_core):
      meta['nch'][b][h]   chunks for block b, half h (same on all cores)
      meta['off16'][b][h] idx-tile int16-column offset of that bucket
      meta['offch'][b][h] chunk offset (for the lrw tile)
      meta['totch']       total chunks
      meta['idxcols']     int16 columns of the idx tensor
    per_core[c] = dict(idx=[128, idxcols] int16, lrw=[128, totch*2] bf16)
    """
    nc_, npad, nb = cfg.n_cores, cfg.npad, cfg.nblocks
    npc = cfg.nodes_per_core
    rows_a = cfg.rows_a
    assert cfg.tab_a <= 32767 and cfg.tab_b <= 32767

    core_of = edge_row // npc
    lr_all = edge_row - core_of * npc          # local dest row on owner core
    cc_of = edge_col // npc                    # core owning the source col
    cl_all = edge_col - cc_of * npc            # local source row
    half_all = (cl_all >= rows_a).astype(np.int64)
    # index within the half table
    tidx_all = np.where(
        half_all == 0,
        cc_of * rows_a + cl_all,
        cc_of * cfg.rows_b + (cl_all - rows_a),
    )

    counts = np.zeros((nc_, nb, 2), np.int64)
    per = {}
    for c in range(nc_):
        m = core_of == c
        lr, ti, hf, w = lr_all[m], tidx_all[m], half_all[m], edge_weight[m]
        blk = lr // 128
        order = np.lexsort((ti, hf, blk))  # sort by (block, half, table idx)
        per[c] = (lr[order], ti[order], hf[order], w[order], blk[order])
        np.add.at(counts[c], (blk, hf), 1)

    chunks_bh = np.ceil(counts / 128.0).astype(np.int64).max(axis=0)  # [nb,2]
    chunks_bh = np.maximum(chunks_bh, 1)

    nch = [[int(chunks_bh[b, h]) for h in (0, 1)] for b in range(nb)]
    off16 = [[0, 0] for _ in range(nb)]
    offch = [[0, 0] for _ in range(nb)]
    tot16 = 0
    totch = 0
    for b in range(nb):
        for h in (0, 1):
            off16[b][h] = tot16
            offch[b][h] = totch
            tot16 += nch[b][h] * 8  # 128 idx per chunk -> 8 int16 cols
            totch += nch[b][h]

    meta = dict(nch=nch, off16=off16, offch=offch, totch=totch, idxcols=tot16)

    per_core = []
    for c in range(nc_):
        lr, ti, hf, w, blk = per[c]
        idx_flat = np.zeros(tot16 * 16, np.int16)
        lr_tab = np.zeros((128, totch), BF16)
        w_tab = np.zeros((128, totch), BF16)
        bucket_no = 0
        p = 0  # cursor into sorted edge stream
        for b in range(nb):
            for h in (0, 1):
                q = p
                while q < len(blk) and blk[q] == b and hf[q] == h:
                    q += 1
                e_ti, e_lr, e_w = ti[p:q], lr[p:q], w[p:q]
                p = q
                n = len(e_ti)
                nslots = nch[b][h] * 128
                pad = nslots - n
                # trailing pads: -1 (ucode trims them) except the first few
                # buckets, whose gather tiles read never-written SBUF.
                pad_idx = 0
                bucket_no += 1
                ti_pad = np.concatenate(
                    [e_ti, np.full(pad, pad_idx, np.int64)]
                )
                i_in = np.arange(nslots)
                base16 = off16[b][h]
                idx_flat[(base16 + i_in // 16) * 16 + (i_in % 16)] = ti_pad.astype(
                    np.int16
                )
                if n > 0:
                    j0 = offch[b][h]
                    i_e = np.arange(n)
                    jj = j0 + i_e // 128
                    ss = i_e % 128
                    lr_tab[ss, jj] = (e_lr - b * 128).astype(BF16)
                    w_tab[ss, jj] = e_w.astype(BF16)
        idx_mat = idx_flat.reshape(tot16, 16).T  # [16, idxcols]
        idx_mat = np.tile(idx_mat, (8, 1))       # replicate to 128 partitions
        per_core.append(
            dict(
                idx=np.ascontiguousarray(idx_mat),
                lrt=np.ascontiguousarray(lr_tab),
                wt=np.ascontiguousarray(w_tab),
            )
        )
    return meta, per_core


def prep_inputs(cfg: Cfg, inputs):
    f = inputs["features"].astype(np.float32)
    meta, per_edge = edge_structure(
        cfg,
        inputs["edge_row"].astype(np.int64),
        inputs["edge_col"].astype(np.int64),
        inputs["edge_weight"].astype(np.float32),
    )
    kin = cfg.in_dim // 128
    k1 = cfg.h1 // 128
    k2 = cfg.h2 // 128

    def wlayout(w, kt):
        K, M = w.shape
        return (
            w.reshape(kt, 128, M).transpose(1, 0, 2).reshape(128, kt * M)
        ).astype(BF16)

    w1 = wlayout(inputs["W_lin1"].astype(np.float32), kin)
    wg1 = wlayout(inputs["W_g1"].astype(np.float32), k1)
    wg2 = wlayout(inputs["W_g2"].astype(np.float32), k2)
    wl2 = wlayout(inputs["W_lin2"].astype(np.float32), k2)
    b1 = inputs["b_lin1"].astype(np.float32).reshape(kin, 128).T.copy()
    bg1 = inputs["b_g1"].astype(BF16).reshape(1, cfg.h2)
    bg2 = inputs["b_g2"].astype(BF16).reshape(1, cfg.h2)
    bl2 = inputs["b_lin2"].astype(BF16).reshape(1, cfg.out_dim)

    npc, npad = cfg.nodes_per_core, cfg.npad
    in_maps = []
    for c in range(cfg.n_cores):
        lo = c * npc
        hi = min((c + 1) * npc, cfg.n_nodes)
        xc = np.zeros((npad, cfg.in_dim), np.float32)
        xc[: hi - lo] = f[lo:hi]
        xt = (
            xc.T.reshape(kin, 128, npad)
            .transpose(1, 0, 2)
            .reshape(128, kin * npad)
        ).astype(BF16)
        in_maps.append(
            {
                "xt": np.ascontiguousarray(xt),
                "w1": w1,
                "wg1": wg1,
                "wg2": wg2,
                "wl2": wl2,
                "b1": b1,
                "bg1": bg1,
                "bg2": bg2,
                "bl2": bl2,
                "idx": per_edge[c]["idx"],
                "lrt": per_edge[c]["lrt"],
                "wt": per_edge[c]["wt"],
            }
        )
    return meta, in_maps


# ---------------------------------------------------------------- kernel IR


def build(cfg: Cfg, meta):
    nc = bacc.Bacc(
        "TRN2",
        target_bir_lowering=False,
        debug=False,
        num_devices=cfg.n_cores,
        num_swdge_queues=4,
    )
    bf = mybir.dt.bfloat16
    f32 = mybir.dt.float32
    i16 = mybir.dt.int16
    kin = cfg.in_dim // 128
    k1 = cfg.h1 // 128
    k2 = cfg.h2 // 128
    npad, nb, H2, OUT = cfg.npad, cfg.nblocks, cfg.h2, cfg.out_dim
    SB = cfg.split_block
    totch = meta["totch"]
    nch = meta["nch"]
    off16 = meta["off16"]
    offch = meta["offch"]

    xt_d = nc.dram_tensor("xt", [128, kin * npad], bf, kind="ExternalInput").ap()
    w1_d = nc.dram_tensor("w1", [128, kin * cfg.h1], bf, kind="ExternalInput").ap()
    wg1_d = nc.dram_tensor("wg1", [128, k1 * H2], bf, kind="ExternalInput").ap()
    wg2_d = nc.dram_tensor("wg2", [128, k2 * H2], bf, kind="ExternalInput").ap()
    wl2_d = nc.dram_tensor("wl2", [128, k2 * OUT], bf, kind="ExternalInput").ap()
    b1_d = nc.dram_tensor("b1", [128, kin], f32, kind="ExternalInput").ap()
    bg1_d = nc.dram_tensor("bg1", [1, H2], bf, kind="ExternalInput").ap()
    bg2_d = nc.dram_tensor("bg2", [1, H2], bf, kind="ExternalInput").ap()
    bl2_d = nc.dram_tensor("bl2", [1, OUT], bf, kind="ExternalInput").ap()
    idx_d = nc.dram_tensor(
        "idx", [128, meta["idxcols"]], i16, kind="ExternalInput"
    ).ap()
    lrt_d = nc.dram_tensor("lrt", [128, totch], bf, kind="ExternalInput").ap()
    wt_d = nc.dram_tensor("wt", [128, totch], bf, kind="ExternalInput").ap()
    y_d = nc.dram_tensor("y", [npad, OUT], f32, kind="ExternalOutput").ap()

    g1locA = nc.dram_tensor("g1locA", [cfg.rows_a, H2], bf).ap()
    g1locB = nc.dram_tensor("g1locB", [cfg.rows_b, H2], bf).ap()
    g2locA = nc.dram_tensor("g2locA", [cfg.rows_a, H2], bf).ap()
    g2locB = nc.dram_tensor("g2locB", [cfg.rows_b, H2], bf).ap()
    g1tabA = nc.dram_tensor("g1tabA", [cfg.tab_a, H2], bf, addr_space="Shared").ap()
    g1tabB = nc.dram_tensor("g1tabB", [cfg.tab_b, H2], bf, addr_space="Shared").ap()
    g2tabA = nc.dram_tensor("g2tabA", [cfg.tab_a, H2], bf, addr_space="Shared").ap()
    g2tabB = nc.dram_tensor("g2tabB", [cfg.tab_b, H2], bf, addr_space="Shared").ap()

    rg = [list(range(cfg.n_cores))]
    qctr = [0]  # round-robin SWDGE queue counter

    def fire_ag(src, dst):
        nc.gpsimd.collective_compute(
            "AllGather",
            mybir.AluOpType.bypass,
            replica_groups=rg,
            ins=[src[:, :]],
            outs=[dst[:, :]],
        )

    with tile.TileContext(nc) as tc:
        with ExitStack() as top:
            const = top.enter_context(tc.tile_pool(name="const", bufs=1))
            w1_s = const.tile([128, kin * cfg.h1], bf)
            nc.sync.dma_start(w1_s[:], w1_d[:, :])
            wg1_s = const.tile([128, k1 * H2], bf)
            nc.sync.dma_start(wg1_s[:], wg1_d[:, :])
            wg2_s = const.tile([128, k2 * H2], bf)
            nc.sync.dma_start(wg2_s[:], wg2_d[:, :])
            wl2_s = const.tile([128, k2 * OUT], bf)
            nc.sync.dma_start(wl2_s[:], wl2_d[:, :])
            b1_s = const.tile([128, kin], f32)
            nc.sync.dma_start(b1_s[:], b1_d[:, :])
            bg1_s = const.tile([1, H2], bf)
            nc.sync.dma_start(bg1_s[:], bg1_d[:, :])
            bg2_s = const.tile([1, H2], bf)
            nc.sync.dma_start(bg2_s[:], bg2_d[:, :])
            bl2_s = const.tile([1, OUT], bf)
            nc.sync.dma_start(bl2_s[:], bl2_d[:, :])
            idx_s = const.tile([128, meta["idxcols"]], i16)
            nc.sync.dma_start(idx_s[:], idx_d[:, :])
            lrt_s = const.tile([128, totch], bf)
            nc.sync.dma_start(lrt_s[:], lrt_d[:, :])
            wt_s = const.tile([128, totch], bf)
            nc.sync.dma_start(wt_s[:], wt_d[:, :])
            iota_i = const.tile([128, 128], i16)
            nc.gpsimd.iota(iota_i[:], pattern=[[1, 128]], base=0, channel_multiplier=0)
            iota_b = const.tile([128, 128], bf)
            nc.vector.tensor_copy(iota_b[:], iota_i[:])
            ident = const.tile([128, 128], bf)
            make_identity(nc, ident[:])
            ones_t = const.tile([1, 128], bf)
            nc.gpsimd.memset(ones_t[:], 1.0)

            # ---------------- L1 + L2a, slab-streamed; fire AG1 chunks early
            slabs = []
            b0 = 0
            while b0 < nb:
                b1e = min(b0 + cfg.slab_blocks, nb)
                slabs.append((b0, b1e))
                b0 = b1e

            with ExitStack() as pl1:
                xp = pl1.enter_context(tc.tile_pool(name="xt", bufs=2))
                hp = pl1.enter_context(tc.tile_pool(name="h1s", bufs=2))
                ps1 = pl1.enter_context(
                    tc.tile_pool(name="ps1", bufs=4, space="PSUM")
                )
                ps2 = pl1.enter_context(
                    tc.tile_pool(name="ps2", bufs=2, space="PSUM")
                )
                gp1 = pl1.enter_context(tc.tile_pool(name="g1t", bufs=3))
                for (bs, be) in slabs:
                    a = bs * 128
                    S = (be - bs) * 128
                    xs = xp.tile([128, kin, S], bf, tag="x")
                    for kt in range(kin):
                        nc.sync.dma_start(
                            xs[:, kt, :], xt_d[:, kt * npad + a : kt * npad + a + S]
                        )
                    h1s = hp.tile([128, k1, S], bf, tag="h")
                    for f1t in range(k1):
                        ps = ps1.tile([128, S], f32, tag="ps")
                        for kt in range(kin):
                            nc.tensor.matmul(
                                ps[:],
                                lhsT=w1_s[
                                    :,
                                    kt * cfg.h1 + f1t * 128 : kt * cfg.h1
                                    + f1t * 128
                                    + 128,
                                ],
                                rhs=xs[:, kt, :],
                                start=(kt == 0),
                                stop=(kt == kin - 1),
                            )
                        nc.scalar.activation(
                            h1s[:, f1t, :],
                            ps[:],
                            AF.Sigmoid,
                            bias=b1_s[:, f1t : f1t + 1],
                        )
                    for b in range(bs, be):
                        o = (b - bs) * 128
                        ps = ps2.tile([128, H2], f32, tag="ps")
                        for kt in range(k1):
                            nc.tensor.matmul(
                                ps[:],
                                lhsT=h1s[:, kt, o : o + 128],
                                rhs=wg1_s[:, kt * H2 : (kt + 1) * H2],
                                start=(kt == 0),
                                stop=(kt == k1 - 1),
                            )
                        g1t = gp1.tile([128, H2], bf, tag="g1")
                        nc.scalar.activation(g1t[:], ps[:], AF.Copy)
                        if b < SB:
                            nc.sync.dma_start(
                                g1locA[b * 128 : (b + 1) * 128, :], g1t[:]
                            )
                        else:
                            bb = b - SB
                            nc.sync.dma_start(
                                g1locB[bb * 128 : (bb + 1) * 128, :], g1t[:]
                            )
                    if be == SB:
                        fire_ag(g1locA, g1tabA)
                assert SB in [be for (_, be) in slabs], "split must be slab-aligned"
                fire_ag(g1locB, g1tabB)

            # Dependency-free PE filler: keeps the HAM activity window hot
            # across phase transitions where the PE would otherwise idle
            # waiting for an AllGather to land (idle >3.4us drops the PE
            # clock from 2.4 to 1.2 GHz for the next stretch of work).
            fillp = top.enter_context(
                tc.tile_pool(name="fill", bufs=1, space="PSUM")
            )

            def pe_filler(n_mm, tag):
                fps = fillp.tile([128, 256], f32, tag="f", name=f"f{tag}")
                for i in range(n_mm):
                    nc.tensor.matmul(
                        fps[:],
                        lhsT=w1_s[:, :128],
                        rhs=w1_s[:, :256],
                        start=True,
                        stop=True,
                    )

            # ---------------- shared spmm machinery (pools shared across
            # both layers so recycled gather tiles always hold prior gather
            # data -- required for the trailing -1 idx-pad trim)
            nch_max = max(max(r) for r in nch)
            nctot_max = max(r[0] + r[1] for r in nch)
            gp = top.enter_context(tc.tile_pool(name="gath", bufs=11))
            pp = top.enter_context(tc.tile_pool(name="pm", bufs=5))
            sp = top.enter_context(tc.tile_pool(name="psmm", bufs=4, space="PSUM"))

            def spmm_layer(ctx, tabA, tabB, brow, out_cb, tag):
                for b in range(nb):
                    tiles = {}
                    for h in (0, 1):
                        n = nch[b][h]
                        t = gp.tile([128, nch_max, H2], bf, tag="g")
                        tab = tabA if h == 0 else tabB
                        for lo in range(0, n, 15):
                            ns = min(15, n - lo)
                            o16 = off16[b][h] + lo * 8
                            nc.gpsimd.dma_gather(
                                out_ap=t[:, lo : lo + ns, :],
                                in_ap=tab[:, :],
                                idxs_ap=idx_s[:, o16 : o16 + ns * 8],
                                num_idxs=ns * 128,
                                num_idxs_reg=ns * 128,
                                elem_size=H2,
                                single_packet=False,
                                queue_num=qctr[0] % 4,
                            )
                            qctr[0] += 1
                        tiles[h] = t
                    nctot = nch[b][0] + nch[b][1]
                    j0 = offch[b][0]
                    pt = pp.tile([128, nctot_max, 128], bf, tag="p")
                    nc.vector.tensor_tensor(
                        pt[:, :nctot, :],
                        iota_b[:].unsqueeze(1).broadcast_to((128, nctot, 128)),
                        lrt_s[:, j0 : j0 + nctot]
                        .unsqueeze(2)
                        .broadcast_to((128, nctot, 128)),
                        ALU.is_equal,
                    )
                    nc.vector.tensor_tensor(
                        pt[:, :nctot, :],
                        pt[:, :nctot, :],
                        wt_s[:, j0 : j0 + nctot]
                        .unsqueeze(2)
                        .broadcast_to((128, nctot, 128)),
                        ALU.mult,
                    )
                    ps = sp.tile([128, H2], f32, tag="ps", name=f"psmm{tag}_{b}")
                    first = True
                    for h in (0, 1):
                        for j in range(nch[b][h]):
                            jl = (offch[b][h] - offch[b][0]) + j
                            nc.tensor.matmul(
                                ps[:],
                                lhsT=pt[:, jl, :],
                                rhs=tiles[h][:, j, :],
                                start=first,
                                stop=False,
                            )
                            first = False
                    nc.tensor.matmul(
                        ps[:],
                        lhsT=ones_t[:1, :],
                        rhs=brow[:1, :],
                        start=first,
                        stop=True,
                    )
                    out_cb(b, ps)

            # ---------------- spmm1 + L3a fused per block; fire AG2 chunks
            with ExitStack() as s1:
                tps3 = s1.enter_context(
                    tc.tile_pool(name="tps3", bufs=1, space="PSUM")
                )
                psp3 = s1.enter_context(
                    tc.tile_pool(name="ps3", bufs=2, space="PSUM")
                )
                tp3 = s1.enter_context(tc.tile_pool(name="l3t", bufs=3))

                def cb1(b, psum):
                    h2t = tp3.tile([128, H2], bf, tag="h2")
                    nc.scalar.activation(h2t[:], psum[:], AF.Relu)
                    h2T = tp3.tile([128, k2, 128], bf, tag="h2T")
                    for kt in range(k2):
                        ptt = tps3.tile([128, 128], bf, tag="pt")
                        nc.tensor.transpose(
                            ptt[:], h2t[:, kt * 128 : (kt + 1) * 128], ident[:]
                        )
                        nc.scalar.activation(h2T[:, kt, :], ptt[:], AF.Copy)
                    ps3 = psp3.tile([128, H2], f32, tag="ps")
                    for kt in range(k2):
                        nc.tensor.matmul(
                            ps3[:],
                            lhsT=h2T[:, kt, :],
                            rhs=wg2_s[:, kt * H2 : (kt + 1) * H2],
                            start=(kt == 0),
                            stop=(kt == k2 - 1),
                        )
                    g2t = tp3.tile([128, H2], bf, tag="g2")
                    nc.scalar.activation(g2t[:], ps3[:], AF.Copy)
                    if b < SB:
                        nc.sync.dma_start(g2locA[b * 128 : (b + 1) * 128, :], g2t[:])
                        if b == SB - 1:
                            fire_ag(g2locA, g2tabA)
                    else:
                        bb = b - SB
                        nc.sync.dma_start(
                            g2locB[bb * 128 : (bb + 1) * 128, :], g2t[:]
                        )
                        if b == nb - 1:
                            fire_ag(g2locB, g2tabB)

                pe_filler(150, "a")
                spmm_layer(s1, g1tabA, g1tabB, bg1_s, cb1, "a")

            # ---------------- spmm2 + L4 fused per block
            with ExitStack() as s2:
                tps4 = s2.enter_context(
                    tc.tile_pool(name="tps4", bufs=1, space="PSUM")
                )
                psp4 = s2.enter_context(
                    tc.tile_pool(name="ps4", bufs=2, space="PSUM")
                )
                tp4 = s2.enter_context(tc.tile_pool(name="l4t", bufs=3))

                def cb2(b, psum):
                    h3t = tp4.tile([128, H2], bf, tag="h3")
                    nc.scalar.activation(h3t[:], psum[:], AF.Relu)
                    h3T = tp4.tile([128, k2, 128], bf, tag="h3T")
                    for kt in range(k2):
                        ptt = tps4.tile([128, 128], bf, tag="pt")
                        nc.tensor.transpose(
                            ptt[:], h3t[:, kt * 128 : (kt + 1) * 128], ident[:]
                        )
                        nc.scalar.activation(h3T[:, kt, :], ptt[:], AF.Copy)
                    ps4 = psp4.tile([128, OUT], f32, tag="ps")
                    for kt in range(k2):
                        nc.tensor.matmul(
                            ps4[:],
                            lhsT=h3T[:, kt, :],
                            rhs=wl2_s[:, kt * OUT : (kt + 1) * OUT],
                            start=(kt == 0),
                            stop=False,
                        )
                    nc.tensor.matmul(
                        ps4[:],
                        lhsT=ones_t[:1, :],
                        rhs=bl2_s[:1, :],
                        start=False,
                        stop=True,
                    )
                    yt = tp4.tile([128, OUT], f32, tag="y")
                    nc.scalar.activation(yt[:], ps4[:], AF.Copy)
                    nc.sync.dma_start(y_d[b * 128 : (b + 1) * 128, :], yt[:])

                pe_filler(100, "b")
                spmm_layer(s2, g2tabA, g2tabB, bg2_s, cb2, "b")

    nc.compile()
    return nc


# ---------------------------------------------------------------- driver

_CACHE = {}


def run(inputs, cfg: Cfg = FULL, trace=False, tmpdir=None):
    meta, in_maps = prep_inputs(cfg, inputs)
    key = (cfg, meta["totch"], meta["idxcols"])
    if key not in _CACHE:
        _CACHE[key] = build(cfg, meta)
    nc = _CACHE[key]
    res = run_bass_kernel_spmd(
        nc,
        in_maps,
        core_ids=list(range(cfg.n_cores)),
        trace=trace,
        tmpdir=tmpdir,
    )
    npc = cfg.nodes_per_core
    out = np.empty((cfg.n_nodes, cfg.out_dim), np.float32)
    for c in range(cfg.n_cores):
        lo = c * npc
        hi = min((c + 1) * npc, cfg.n_nodes)
        out[lo:hi] = res.results[c]["y"][: hi - lo]
    return out, res


def kernel(**inputs) -> np.ndarray:
    out, _ = run(inputs, FULL, trace=False)
    return out


# revision 30
# speedup vs baseline: 1.0216x; 1.0064x over previous
"""GNN message-passing kernel for 8 Trainium2 NeuronCores (Bass/Tile).

Takes FULL inputs, shards nodes across 8 cores internally, runs the
4-layer GNN (dense -> spmm -> spmm -> dense) with two bf16 AllGathers
of the hidden node table (each split in two chunks, fired as soon as
the producing blocks finish), and PE-matmul-based weighted segment
sums.  The one-hot selector matrices are built ON-CHIP on the vector
engine from per-edge (local-row, weight) pairs; the row gathers are
spread round-robin over 4 SWDGE queues so descriptor generation runs
on all four GpSimd Q7 core-pairs in parallel.
"""

import math
from contextlib import ExitStack
from dataclasses import dataclass

import ml_dtypes
import numpy as np

import concourse.bass as bass
import concourse.mybir as mybir
import concourse.tile as tile
from concourse import bacc
from concourse.bass_utils import run_bass_kernel_spmd
from concourse.masks import make_identity

BF16 = ml_dtypes.bfloat16
AF = mybir.ActivationFunctionType
ALU = mybir.AluOpType


@dataclass(frozen=True)
class Cfg:
    n_nodes: int = 50000
    n_edges: int = 800000
    in_dim: int = 512
    h1: int = 512
    h2: int = 256
    out_dim: int = 128
    n_cores: int = 8
    split_block: int = 28  # blocks [0, 28) -> half A, [28, 49) -> half B
    slab_blocks: int = 4   # L1/L2a node-slab width in 128-blocks

    @property
    def nodes_per_core(self):
        return self.n_nodes // self.n_cores  # 6250

    @property
    def npad(self):
        return math.ceil(self.nodes_per_core / 128) * 128  # 6272

    @property
    def nblocks(self):
        return self.npad // 128  # 49

    @property
    def rows_a(self):
        return self.split_block * 128  # 3584

    @property
    def rows_b(self):
        return self.npad - self.rows_a  # 2688

    @property
    def tab_a(self):
        return self.rows_a * self.n_cores  # 28672

    @property
    def tab_b(self):
        return self.rows_b * self.n_cores  # 21504


FULL = Cfg()


# ---------------------------------------------------------------- host prep


def edge_structure(cfg: Cfg, edge_row, edge_col, edge_weight):
    """Bucket edges per (core, row-block, table-half); uniform chunk counts.

    Returns (meta, per_core):
      meta['nch'][b][h]   chunks for block b, half h (same on all cores)
      meta['off16'][b][h] idx-tile int16-column offset of that bucket
      meta['offch'][b][h] chunk offset (for the lrw tile)
      meta['totch']       total chunks
      meta['idxcols']     int16 columns of the idx tensor
    per_core[c] = dict(idx=[128, idxcols] int16, lrw=[128, totch*2] bf16)
    """
    nc_, npad, nb = cfg.n_cores, cfg.npad, cfg.nblocks
    npc = cfg.nodes_per_core
    rows_a = cfg.rows_a
    assert cfg.tab_a <= 32767 and cfg.tab_b <= 32767

    core_of = edge_row // npc
    lr_all = edge_row - core_of * npc          # local dest row on owner core
    cc_of = edge_col // npc                    # core owning the source col
    cl_all = edge_col - cc_of * npc            # local source row
    half_all = (cl_all >= rows_a).astype(np.int64)
    # index within the half table
    tidx_all = np.where(
        half_all == 0,
        cc_of * rows_a + cl_all,
        cc_of * cfg.rows_b + (cl_all - rows_a),
    )

    counts = np.zeros((nc_, nb, 2), np.int64)
    per = {}
    for c in range(nc_):
        m = core_of == c
        lr, ti, hf, w = lr_all[m], tidx_all[m], half_all[m], edge_weight[m]
        blk = lr // 128
        order = np.lexsort((ti, hf, blk))  # sort by (block, half, table idx)
        per[c] = (lr[order], ti[order], hf[order], w[order], blk[order])
        np.add.at(counts[c], (blk, hf), 1)

    chunks_bh = np.ceil(counts / 128.0).astype(np.int64).max(axis=0)  # [nb,2]
    chunks_bh = np.maximum(chunks_bh, 1)

    nch = [[int(chunks_bh[b, h]) for h in (0, 1)] for b in range(nb)]
    off16 = [[0, 0] for _ in range(nb)]
    offch = [[0, 0] for _ in range(nb)]
    tot16 = 0
    totch = 0
    for b in range(nb):
        for h in (0, 1):
            off16[b][h] = tot16
            offch[b][h] = totch
            tot16 += nch[b][h] * 8  # 128 idx per chunk -> 8 int16 cols
            totch += nch[b][h]

    meta = dict(nch=nch, off16=off16, offch=offch, totch=totch, idxcols=tot16)

    per_core = []
    for c in range(nc_):
        lr, ti, hf, w, blk = per[c]
        idx_flat = np.zeros(tot16 * 16, np.int16)
        lr_tab = np.zeros((128, totch), BF16)
        w_tab = np.zeros((128, totch), BF16)
        bucket_no = 0
        p = 0  # cursor into sorted edge stream
        for b in range(nb):
            for h in (0, 1):
                q = p
                while q < len(blk) and blk[q] == b and hf[q] == h:
                    q += 1
                e_ti, e_lr, e_w = ti[p:q], lr[p:q], w[p:q]
                p = q
                n = len(e_ti)
                nslots = nch[b][h] * 128
                pad = nslots - n
                # trailing pads: -1 (ucode trims them) except the first few
                # buckets, whose gather tiles read never-written SBUF.
                pad_idx = 0
                bucket_no += 1
                ti_pad = np.concatenate(
                    [e_ti, np.full(pad, pad_idx, np.int64)]
                )
                i_in = np.arange(nslots)
                base16 = off16[b][h]
                idx_flat[(base16 + i_in // 16) * 16 + (i_in % 16)] = ti_pad.astype(
                    np.int16
                )
                if n > 0:
                    j0 = offch[b][h]
                    i_e = np.arange(n)
                    jj = j0 + i_e // 128
                    ss = i_e % 128
                    lr_tab[ss, jj] = (e_lr - b * 128).astype(BF16)
                    w_tab[ss, jj] = e_w.astype(BF16)
        idx_mat = idx_flat.reshape(tot16, 16).T  # [16, idxcols]
        idx_mat = np.tile(idx_mat, (8, 1))       # replicate to 128 partitions
        per_core.append(
            dict(
                idx=np.ascontiguousarray(idx_mat),
                lrt=np.ascontiguousarray(lr_tab),
                wt=np.ascontiguousarray(w_tab),
            )
        )
    return meta, per_core


def prep_inputs(cfg: Cfg, inputs):
    f = inputs["features"].astype(np.float32)
    meta, per_edge = edge_structure(
        cfg,
        inputs["edge_row"].astype(np.int64),
        inputs["edge_col"].astype(np.int64),
        inputs["edge_weight"].astype(np.float32),
    )
    kin = cfg.in_dim // 128
    k1 = cfg.h1 // 128
    k2 = cfg.h2 // 128

    def wlayout(w, kt):
        K, M = w.shape
        return (
            w.reshape(kt, 128, M).transpose(1, 0, 2).reshape(128, kt * M)
        ).astype(BF16)

    w1 = wlayout(inputs["W_lin1"].astype(np.float32), kin)
    wg1 = wlayout(inputs["W_g1"].astype(np.float32), k1)
    wg2 = wlayout(inputs["W_g2"].astype(np.float32), k2)
    wl2 = wlayout(inputs["W_lin2"].astype(np.float32), k2)
    b1 = inputs["b_lin1"].astype(np.float32).reshape(kin, 128).T.copy()
    bg1 = inputs["b_g1"].astype(BF16).reshape(1, cfg.h2)
    bg2 = inputs["b_g2"].astype(BF16).reshape(1, cfg.h2)
    bl2 = inputs["b_lin2"].astype(BF16).reshape(1, cfg.out_dim)

    npc, npad = cfg.nodes_per_core, cfg.npad
    in_maps = []
    for c in range(cfg.n_cores):
        lo = c * npc
        hi = min((c + 1) * npc, cfg.n_nodes)
        xc = np.zeros((npad, cfg.in_dim), np.float32)
        xc[: hi - lo] = f[lo:hi]
        xt = (
            xc.T.reshape(kin, 128, npad)
            .transpose(1, 0, 2)
            .reshape(128, kin * npad)
        ).astype(BF16)
        in_maps.append(
            {
                "xt": np.ascontiguousarray(xt),
                "w1": w1,
                "wg1": wg1,
                "wg2": wg2,
                "wl2": wl2,
                "b1": b1,
                "bg1": bg1,
                "bg2": bg2,
                "bl2": bl2,
                "idx": per_edge[c]["idx"],
                "lrt": per_edge[c]["lrt"],
                "wt": per_edge[c]["wt"],
            }
        )
    return meta, in_maps


# ---------------------------------------------------------------- kernel IR


def build(cfg: Cfg, meta):
    nc = bacc.Bacc(
        "TRN2",
        target_bir_lowering=False,
        debug=False,
        num_devices=cfg.n_cores,
        num_swdge_queues=4,
    )
    bf = mybir.dt.bfloat16
    f32 = mybir.dt.float32
    i16 = mybir.dt.int16
    kin = cfg.in_dim // 128
    k1 = cfg.h1 // 128
    k2 = cfg.h2 // 128
    npad, nb, H2, OUT = cfg.npad, cfg.nblocks, cfg.h2, cfg.out_dim
    SB = cfg.split_block
    totch = meta["totch"]
    nch = meta["nch"]
    off16 = meta["off16"]
    offch = meta["offch"]

    xt_d = nc.dram_tensor("xt", [128, kin * npad], bf, kind="ExternalInput").ap()
    w1_d = nc.dram_tensor("w1", [128, kin * cfg.h1], bf, kind="ExternalInput").ap()
    wg1_d = nc.dram_tensor("wg1", [128, k1 * H2], bf, kind="ExternalInput").ap()
    wg2_d = nc.dram_tensor("wg2", [128, k2 * H2], bf, kind="ExternalInput").ap()
    wl2_d = nc.dram_tensor("wl2", [128, k2 * OUT], bf, kind="ExternalInput").ap()
    b1_d = nc.dram_tensor("b1", [128, kin], f32, kind="ExternalInput").ap()
    bg1_d = nc.dram_tensor("bg1", [1, H2], bf, kind="ExternalInput").ap()
    bg2_d = nc.dram_tensor("bg2", [1, H2], bf, kind="ExternalInput").ap()
    bl2_d = nc.dram_tensor("bl2", [1, OUT], bf, kind="ExternalInput").ap()
    idx_d = nc.dram_tensor(
        "idx", [128, meta["idxcols"]], i16, kind="ExternalInput"
    ).ap()
    lrt_d = nc.dram_tensor("lrt", [128, totch], bf, kind="ExternalInput").ap()
    wt_d = nc.dram_tensor("wt", [128, totch], bf, kind="ExternalInput").ap()
    y_d = nc.dram_tensor("y", [npad, OUT], f32, kind="ExternalOutput").ap()

    g1locA = nc.dram_tensor("g1locA", [cfg.rows_a, H2], bf).ap()
    g1locB = nc.dram_tensor("g1locB", [cfg.rows_b, H2], bf).ap()
    g2locA = nc.dram_tensor("g2locA", [cfg.rows_a, H2], bf).ap()
    g2locB = nc.dram_tensor("g2locB", [cfg.rows_b, H2], bf).ap()
    g1tabA = nc.dram_tensor("g1tabA", [cfg.tab_a, H2], bf, addr_space="Shared").ap()
    g1tabB = nc.dram_tensor("g1tabB", [cfg.tab_b, H2], bf, addr_space="Shared").ap()
    g2tabA = nc.dram_tensor("g2tabA", [cfg.tab_a, H2], bf, addr_space="Shared").ap()
    g2tabB = nc.dram_tensor("g2tabB", [cfg.tab_b, H2], bf, addr_space="Shared").ap()

    rg = [list(range(cfg.n_cores))]
    qctr = [0]  # round-robin SWDGE queue counter

    def fire_ag(src, dst):
        nc.gpsimd.collective_compute(
            "AllGather",
            mybir.AluOpType.bypass,
            replica_groups=rg,
            ins=[src[:, :]],
            outs=[dst[:, :]],
        )

    with tile.TileContext(nc) as tc:
        with ExitStack() as top:
            const = top.enter_context(tc.tile_pool(name="const", bufs=1))
            w1_s = const.tile([128, kin * cfg.h1], bf)
            nc.sync.dma_start(w1_s[:], w1_d[:, :])
            wg1_s = const.tile([128, k1 * H2], bf)
            nc.sync.dma_start(wg1_s[:], wg1_d[:, :])
            wg2_s = const.tile([128, k2 * H2], bf)
            nc.sync.dma_start(wg2_s[:], wg2_d[:, :])
            wl2_s = const.tile([128, k2 * OUT], bf)
            nc.sync.dma_start(wl2_s[:], wl2_d[:, :])
            b1_s = const.tile([128, kin], f32)
            nc.sync.dma_start(b1_s[:], b1_d[:, :])
            bg1_s = const.tile([1, H2], bf)
            nc.sync.dma_start(bg1_s[:], bg1_d[:, :])
            bg2_s = const.tile([1, H2], bf)
            nc.sync.dma_start(bg2_s[:], bg2_d[:, :])
            bl2_s = const.tile([1, OUT], bf)
            nc.sync.dma_start(bl2_s[:], bl2_d[:, :])
            idx_s = const.tile([128, meta["idxcols"]], i16)
            nc.sync.dma_start(idx_s[:], idx_d[:, :])
            lrt_s = const.tile([128, totch], bf)
            nc.sync.dma_start(lrt_s[:], lrt_d[:, :])
            wt_s = const.tile([128, totch], bf)
            nc.sync.dma_start(wt_s[:], wt_d[:, :])
            iota_i = const.tile([128, 128], i16)
            nc.gpsimd.iota(iota_i[:], pattern=[[1, 128]], base=0, channel_multiplier=0)
            iota_b = const.tile([128, 128], bf)
            nc.vector.tensor_copy(iota_b[:], iota_i[:])
            ident = const.tile([128, 128], bf)
            make_identity(nc, ident[:])
            ones_t = const.tile([1, 128], bf)
            nc.gpsimd.memset(ones_t[:], 1.0)

            # ---------------- L1 + L2a, slab-streamed; fire AG1 chunks early
            slabs = []
            b0 = 0
            while b0 < nb:
                b1e = min(b0 + cfg.slab_blocks, nb)
                slabs.append((b0, b1e))
                b0 = b1e

            with ExitStack() as pl1:
                xp = pl1.enter_context(tc.tile_pool(name="xt", bufs=2))
                hp = pl1.enter_context(tc.tile_pool(name="h1s", bufs=2))
                ps1 = pl1.enter_context(
                    tc.tile_pool(name="ps1", bufs=4, space="PSUM")
                )
                ps2 = pl1.enter_context(
                    tc.tile_pool(name="ps2", bufs=2, space="PSUM")
                )
                gp1 = pl1.enter_context(tc.tile_pool(name="g1t", bufs=3))
                for (bs, be) in slabs:
                    a = bs * 128
                    S = (be - bs) * 128
                    xs = xp.tile([128, kin, S], bf, tag="x")
                    for kt in range(kin):
                        nc.sync.dma_start(
                            xs[:, kt, :], xt_d[:, kt * npad + a : kt * npad + a + S]
                        )
                    h1s = hp.tile([128, k1, S], bf, tag="h")
                    for f1t in range(k1):
                        ps = ps1.tile([128, S], f32, tag="ps")
                        for kt in range(kin):
                            nc.tensor.matmul(
                                ps[:],
                                lhsT=w1_s[
                                    :,
                                    kt * cfg.h1 + f1t * 128 : kt * cfg.h1
                                    + f1t * 128
                                    + 128,
                                ],
                                rhs=xs[:, kt, :],
                                start=(kt == 0),
                                stop=(kt == kin - 1),
                            )
                        nc.scalar.activation(
                            h1s[:, f1t, :],
                            ps[:],
                            AF.Sigmoid,
                            bias=b1_s[:, f1t : f1t + 1],
                        )
                    for b in range(bs, be):
                        o = (b - bs) * 128
                        ps = ps2.tile([128, H2], f32, tag="ps")
                        for kt in range(k1):
                            nc.tensor.matmul(
                                ps[:],
                                lhsT=h1s[:, kt, o : o + 128],
                                rhs=wg1_s[:, kt * H2 : (kt + 1) * H2],
                                start=(kt == 0),
                                stop=(kt == k1 - 1),
                            )
                        g1t = gp1.tile([128, H2], bf, tag="g1")
                        nc.vector.tensor_copy(g1t[:], ps[:])
                        if b < SB:
                            nc.sync.dma_start(
                                g1locA[b * 128 : (b + 1) * 128, :], g1t[:]
                            )
                        else:
                            bb = b - SB
                            nc.sync.dma_start(
                                g1locB[bb * 128 : (bb + 1) * 128, :], g1t[:]
                            )
                    if be == SB:
                        fire_ag(g1locA, g1tabA)
                assert SB in [be for (_, be) in slabs], "split must be slab-aligned"
                fire_ag(g1locB, g1tabB)

            # Dependency-free PE filler: keeps the HAM activity window hot
            # across phase transitions where the PE would otherwise idle
            # waiting for an AllGather to land (idle >3.4us drops the PE
            # clock from 2.4 to 1.2 GHz for the next stretch of work).
            fillp = top.enter_context(
                tc.tile_pool(name="fill", bufs=1, space="PSUM")
            )

            def pe_filler(n_mm, tag):
                fps = fillp.tile([128, 256], f32, tag="f", name=f"f{tag}")
                for i in range(n_mm):
                    nc.tensor.matmul(
                        fps[:],
                        lhsT=w1_s[:, :128],
                        rhs=w1_s[:, :256],
                        start=True,
                        stop=True,
                    )

            # ---------------- shared spmm machinery (pools shared across
            # both layers so recycled gather tiles always hold prior gather
            # data -- required for the trailing -1 idx-pad trim)
            nch_max = max(max(r) for r in nch)
            nctot_max = max(r[0] + r[1] for r in nch)
            gp = top.enter_context(tc.tile_pool(name="gath", bufs=12))
            pp = top.enter_context(tc.tile_pool(name="pm", bufs=5))
            sp = top.enter_context(tc.tile_pool(name="psmm", bufs=4, space="PSUM"))

            LEAD = 4  # A-half gathers run this many blocks ahead of B-half

            def spmm_layer(ctx, tabA, tabB, brow, out_cb, tag):
                tiles = {}

                def issue(b, h):
                    n = nch[b][h]
                    t = gp.tile([128, nch_max, H2], bf, tag="g")
                    tab = tabA if h == 0 else tabB
                    for lo in range(0, n, 15):
                        ns = min(15, n - lo)
                        o16 = off16[b][h] + lo * 8
                        nc.gpsimd.dma_gather(
                            out_ap=t[:, lo : lo + ns, :],
                            in_ap=tab[:, :],
                            idxs_ap=idx_s[:, o16 : o16 + ns * 8],
                            num_idxs=ns * 128,
                            num_idxs_reg=ns * 128,
                            elem_size=H2,
                            single_packet=False,
                            queue_num=qctr[0] % 4,
                        )
                        qctr[0] += 1
                    tiles[(b, h)] = t

                for b in range(min(LEAD, nb)):
                    issue(b, 0)
                for b in range(nb):
                    if b + LEAD < nb:
                        issue(b + LEAD, 0)
                    issue(b, 1)
                    nctot = nch[b][0] + nch[b][1]
                    j0 = offch[b][0]
                    # (indented under the per-block loop continued below)
                    pt = pp.tile([128, nctot_max, 128], bf, tag="p")
                    nc.vector.tensor_tensor(
                        pt[:, :nctot, :],
                        iota_b[:].unsqueeze(1).broadcast_to((128, nctot, 128)),
                        lrt_s[:, j0 : j0 + nctot]
                        .unsqueeze(2)
                        .broadcast_to((128, nctot, 128)),
                        ALU.is_equal,
                    )
                    nc.vector.tensor_tensor(
                        pt[:, :nctot, :],
                        pt[:, :nctot, :],
                        wt_s[:, j0 : j0 + nctot]
                        .unsqueeze(2)
                        .broadcast_to((128, nctot, 128)),
                        ALU.mult,
                    )
                    ps = sp.tile([128, H2], f32, tag="ps", name=f"psmm{tag}_{b}")
                    first = True
                    for h in (0, 1):
                        for j in range(nch[b][h]):
                            jl = (offch[b][h] - offch[b][0]) + j
                            nc.tensor.matmul(
                                ps[:],
                                lhsT=pt[:, jl, :],
                                rhs=tiles[(b, h)][:, j, :],
                                start=first,
                                stop=False,
                            )
                            first = False
                    nc.tensor.matmul(
                        ps[:],
                        lhsT=ones_t[:1, :],
                        rhs=brow[:1, :],
                        start=first,
                        stop=True,
                    )
                    out_cb(b, ps)

            # ---------------- spmm1 + L3a fused per block; fire AG2 chunks
            with ExitStack() as s1:
                tps3 = s1.enter_context(
                    tc.tile_pool(name="tps3", bufs=1, space="PSUM")
                )
                psp3 = s1.enter_context(
                    tc.tile_pool(name="ps3", bufs=2, space="PSUM")
                )
                tp3 = s1.enter_context(tc.tile_pool(name="l3t", bufs=3))

                def cb1(b, psum):
                    h2t = tp3.tile([128, H2], bf, tag="h2")
                    nc.scalar.activation(h2t[:], psum[:], AF.Relu)
                    h2T = tp3.tile([128, k2, 128], bf, tag="h2T")
                    for kt in range(k2):
                        ptt = tps3.tile([128, 128], bf, tag="pt")
                        nc.tensor.transpose(
                            ptt[:], h2t[:, kt * 128 : (kt + 1) * 128], ident[:]
                        )
                        nc.scalar.activation(h2T[:, kt, :], ptt[:], AF.Copy)
                    ps3 = psp3.tile([128, H2], f32, tag="ps")
                    for kt in range(k2):
                        nc.tensor.matmul(
                            ps3[:],
                            lhsT=h2T[:, kt, :],
                            rhs=wg2_s[:, kt * H2 : (kt + 1) * H2],
                            start=(kt == 0),
                            stop=(kt == k2 - 1),
                        )
                    g2t = tp3.tile([128, H2], bf, tag="g2")
                    nc.scalar.activation(g2t[:], ps3[:], AF.Copy)
                    if b < SB:
                        nc.sync.dma_start(g2locA[b * 128 : (b + 1) * 128, :], g2t[:])
                        if b == SB - 1:
                            fire_ag(g2locA, g2tabA)
                    else:
                        bb = b - SB
                        nc.sync.dma_start(
                            g2locB[bb * 128 : (bb + 1) * 128, :], g2t[:]
                        )
                        if b == nb - 1:
                            fire_ag(g2locB, g2tabB)

                pe_filler(150, "a")
                spmm_layer(s1, g1tabA, g1tabB, bg1_s, cb1, "a")

            # ---------------- spmm2 + L4 fused per block
            with ExitStack() as s2:
                tps4 = s2.enter_context(
                    tc.tile_pool(name="tps4", bufs=1, space="PSUM")
                )
                psp4 = s2.enter_context(
                    tc.tile_pool(name="ps4", bufs=2, space="PSUM")
                )
                tp4 = s2.enter_context(tc.tile_pool(name="l4t", bufs=3))

                def cb2(b, psum):
                    h3t = tp4.tile([128, H2], bf, tag="h3")
                    nc.scalar.activation(h3t[:], psum[:], AF.Relu)
                    h3T = tp4.tile([128, k2, 128], bf, tag="h3T")
                    for kt in range(k2):
                        ptt = tps4.tile([128, 128], bf, tag="pt")
                        nc.tensor.transpose(
                            ptt[:], h3t[:, kt * 128 : (kt + 1) * 128], ident[:]
                        )
                        nc.scalar.activation(h3T[:, kt, :], ptt[:], AF.Copy)
                    ps4 = psp4.tile([128, OUT], f32, tag="ps")
                    for kt in range(k2):
                        nc.tensor.matmul(
                            ps4[:],
                            lhsT=h3T[:, kt, :],
                            rhs=wl2_s[:, kt * OUT : (kt + 1) * OUT],
                            start=(kt == 0),
                            stop=False,
                        )
                    nc.tensor.matmul(
                        ps4[:],
                        lhsT=ones_t[:1, :],
                        rhs=bl2_s[:1, :],
                        start=False,
                        stop=True,
                    )
                    yt = tp4.tile([128, OUT], f32, tag="y")
                    nc.scalar.activation(yt[:], ps4[:], AF.Copy)
                    nc.sync.dma_start(y_d[b * 128 : (b + 1) * 128, :], yt[:])

                pe_filler(100, "b")
                spmm_layer(s2, g2tabA, g2tabB, bg2_s, cb2, "b")

    nc.compile()
    return nc


# ---------------------------------------------------------------- driver

_CACHE = {}


def run(inputs, cfg: Cfg = FULL, trace=False, tmpdir=None):
    meta, in_maps = prep_inputs(cfg, inputs)
    key = (cfg, meta["totch"], meta["idxcols"])
    if key not in _CACHE:
        _CACHE[key] = build(cfg, meta)
    nc = _CACHE[key]
    res = run_bass_kernel_spmd(
        nc,
        in_maps,
        core_ids=list(range(cfg.n_cores)),
        trace=trace,
        tmpdir=tmpdir,
    )
    npc = cfg.nodes_per_core
    out = np.empty((cfg.n_nodes, cfg.out_dim), np.float32)
    for c in range(cfg.n_cores):
        lo = c * npc
        hi = min((c + 1) * npc, cfg.n_nodes)
        out[lo:hi] = res.results[c]["y"][: hi - lo]
    return out, res


def kernel(**inputs) -> np.ndarray:
    out, _ = run(inputs, FULL, trace=False)
    return out


# revision 31
# speedup vs baseline: 1.0359x; 1.0140x over previous
"""GNN message-passing kernel for 8 Trainium2 NeuronCores (Bass/Tile).

Takes FULL inputs, shards nodes across 8 cores internally, runs the
4-layer GNN (dense -> spmm -> spmm -> dense) with two bf16 AllGathers
of the hidden node table (each split in two chunks, fired as soon as
the producing blocks finish), and PE-matmul-based weighted segment
sums.  The one-hot selector matrices are built ON-CHIP on the vector
engine from per-edge (local-row, weight) pairs; the row gathers are
spread round-robin over 4 SWDGE queues so descriptor generation runs
on all four GpSimd Q7 core-pairs in parallel.
"""

import math
from contextlib import ExitStack
from dataclasses import dataclass

import ml_dtypes
import numpy as np

import concourse.bass as bass
import concourse.mybir as mybir
import concourse.tile as tile
from concourse import bacc
from concourse.bass_utils import run_bass_kernel_spmd
from concourse.masks import make_identity

BF16 = ml_dtypes.bfloat16
AF = mybir.ActivationFunctionType
ALU = mybir.AluOpType


@dataclass(frozen=True)
class Cfg:
    n_nodes: int = 50000
    n_edges: int = 800000
    in_dim: int = 512
    h1: int = 512
    h2: int = 256
    out_dim: int = 128
    n_cores: int = 8
    split_block: int = 28  # blocks [0, 28) -> half A, [28, 49) -> half B
    slab_blocks: int = 4   # L1/L2a node-slab width in 128-blocks

    @property
    def nodes_per_core(self):
        return self.n_nodes // self.n_cores  # 6250

    @property
    def npad(self):
        return math.ceil(self.nodes_per_core / 128) * 128  # 6272

    @property
    def nblocks(self):
        return self.npad // 128  # 49

    @property
    def rows_a(self):
        return self.split_block * 128  # 3584

    @property
    def rows_b(self):
        return self.npad - self.rows_a  # 2688

    @property
    def tab_a(self):
        return self.rows_a * self.n_cores  # 28672

    @property
    def tab_b(self):
        return self.rows_b * self.n_cores  # 21504


FULL = Cfg()


# ---------------------------------------------------------------- host prep


def edge_structure(cfg: Cfg, edge_row, edge_col, edge_weight):
    """Bucket edges per (core, row-block, table-half); uniform chunk counts.

    Returns (meta, per_core):
      meta['nch'][b][h]   chunks for block b, half h (same on all cores)
      meta['off16'][b][h] idx-tile int16-column offset of that bucket
      meta['offch'][b][h] chunk offset (for the lrw tile)
      meta['totch']       total chunks
      meta['idxcols']     int16 columns of the idx tensor
    per_core[c] = dict(idx=[128, idxcols] int16, lrw=[128, totch*2] bf16)
    """
    nc_, npad, nb = cfg.n_cores, cfg.npad, cfg.nblocks
    npc = cfg.nodes_per_core
    rows_a = cfg.rows_a
    assert cfg.tab_a <= 32767 and cfg.tab_b <= 32767

    core_of = edge_row // npc
    lr_all = edge_row - core_of * npc          # local dest row on owner core
    cc_of = edge_col // npc                    # core owning the source col
    cl_all = edge_col - cc_of * npc            # local source row
    half_all = (cl_all >= rows_a).astype(np.int64)
    # index within the half table
    tidx_all = np.where(
        half_all == 0,
        cc_of * rows_a + cl_all,
        cc_of * cfg.rows_b + (cl_all - rows_a),
    )

    counts = np.zeros((nc_, nb, 2), np.int64)
    per = {}
    for c in range(nc_):
        m = core_of == c
        lr, ti, hf, w = lr_all[m], tidx_all[m], half_all[m], edge_weight[m]
        blk = lr // 128
        order = np.lexsort((ti, hf, blk))  # sort by (block, half, table idx)
        per[c] = (lr[order], ti[order], hf[order], w[order], blk[order])
        np.add.at(counts[c], (blk, hf), 1)

    chunks_bh = np.ceil(counts / 128.0).astype(np.int64).max(axis=0)  # [nb,2]
    chunks_bh = np.maximum(chunks_bh, 1)

    nch = [[int(chunks_bh[b, h]) for h in (0, 1)] for b in range(nb)]
    off16 = [[0, 0] for _ in range(nb)]
    offch = [[0, 0] for _ in range(nb)]
    tot16 = 0
    totch = 0
    for b in range(nb):
        for h in (0, 1):
            off16[b][h] = tot16
            offch[b][h] = totch
            tot16 += nch[b][h] * 8  # 128 idx per chunk -> 8 int16 cols
            totch += nch[b][h]

    meta = dict(nch=nch, off16=off16, offch=offch, totch=totch, idxcols=tot16)

    per_core = []
    for c in range(nc_):
        lr, ti, hf, w, blk = per[c]
        idx_flat = np.zeros(tot16 * 16, np.int16)
        lr_tab = np.zeros((128, totch), BF16)
        w_tab = np.zeros((128, totch), BF16)
        bucket_no = 0
        p = 0  # cursor into sorted edge stream
        for b in range(nb):
            for h in (0, 1):
                q = p
                while q < len(blk) and blk[q] == b and hf[q] == h:
                    q += 1
                e_ti, e_lr, e_w = ti[p:q], lr[p:q], w[p:q]
                p = q
                n = len(e_ti)
                nslots = nch[b][h] * 128
                pad = nslots - n
                # trailing pads: -1 (ucode trims them) except the first few
                # buckets, whose gather tiles read never-written SBUF.
                pad_idx = 0
                bucket_no += 1
                ti_pad = np.concatenate(
                    [e_ti, np.full(pad, pad_idx, np.int64)]
                )
                i_in = np.arange(nslots)
                base16 = off16[b][h]
                idx_flat[(base16 + i_in // 16) * 16 + (i_in % 16)] = ti_pad.astype(
                    np.int16
                )
                if n > 0:
                    j0 = offch[b][h]
                    i_e = np.arange(n)
                    jj = j0 + i_e // 128
                    ss = i_e % 128
                    lr_tab[ss, jj] = (e_lr - b * 128).astype(BF16)
                    w_tab[ss, jj] = e_w.astype(BF16)
        idx_mat = idx_flat.reshape(tot16, 16).T  # [16, idxcols]
        idx_mat = np.tile(idx_mat, (8, 1))       # replicate to 128 partitions
        per_core.append(
            dict(
                idx=np.ascontiguousarray(idx_mat),
                lrt=np.ascontiguousarray(lr_tab),
                wt=np.ascontiguousarray(w_tab),
            )
        )
    return meta, per_core


def prep_inputs(cfg: Cfg, inputs):
    f = inputs["features"].astype(np.float32)
    meta, per_edge = edge_structure(
        cfg,
        inputs["edge_row"].astype(np.int64),
        inputs["edge_col"].astype(np.int64),
        inputs["edge_weight"].astype(np.float32),
    )
    kin = cfg.in_dim // 128
    k1 = cfg.h1 // 128
    k2 = cfg.h2 // 128

    def wlayout(w, kt):
        K, M = w.shape
        return (
            w.reshape(kt, 128, M).transpose(1, 0, 2).reshape(128, kt * M)
        ).astype(BF16)

    w1 = wlayout(inputs["W_lin1"].astype(np.float32), kin)
    wg1 = wlayout(inputs["W_g1"].astype(np.float32), k1)
    wg2 = wlayout(inputs["W_g2"].astype(np.float32), k2)
    wl2 = wlayout(inputs["W_lin2"].astype(np.float32), k2)
    b1 = inputs["b_lin1"].astype(np.float32).reshape(kin, 128).T.copy()
    bg1 = inputs["b_g1"].astype(BF16).reshape(1, cfg.h2)
    bg2 = inputs["b_g2"].astype(BF16).reshape(1, cfg.h2)
    bl2 = inputs["b_lin2"].astype(BF16).reshape(1, cfg.out_dim)

    npc, npad = cfg.nodes_per_core, cfg.npad
    in_maps = []
    for c in range(cfg.n_cores):
        lo = c * npc
        hi = min((c + 1) * npc, cfg.n_nodes)
        xc = np.zeros((npad, cfg.in_dim), np.float32)
        xc[: hi - lo] = f[lo:hi]
        xt = (
            xc.T.reshape(kin, 128, npad)
            .transpose(1, 0, 2)
            .reshape(128, kin * npad)
        ).astype(BF16)
        in_maps.append(
            {
                "xt": np.ascontiguousarray(xt),
                "w1": w1,
                "wg1": wg1,
                "wg2": wg2,
                "wl2": wl2,
                "b1": b1,
                "bg1": bg1,
                "bg2": bg2,
                "bl2": bl2,
                "idx": per_edge[c]["idx"],
                "lrt": per_edge[c]["lrt"],
                "wt": per_edge[c]["wt"],
            }
        )
    return meta, in_maps


# ---------------------------------------------------------------- kernel IR


def build(cfg: Cfg, meta):
    nc = bacc.Bacc(
        "TRN2",
        target_bir_lowering=False,
        debug=False,
        num_devices=cfg.n_cores,
        num_swdge_queues=4,
    )
    bf = mybir.dt.bfloat16
    f32 = mybir.dt.float32
    i16 = mybir.dt.int16
    kin = cfg.in_dim // 128
    k1 = cfg.h1 // 128
    k2 = cfg.h2 // 128
    npad, nb, H2, OUT = cfg.npad, cfg.nblocks, cfg.h2, cfg.out_dim
    SB = cfg.split_block
    totch = meta["totch"]
    nch = meta["nch"]
    off16 = meta["off16"]
    offch = meta["offch"]

    xt_d = nc.dram_tensor("xt", [128, kin * npad], bf, kind="ExternalInput").ap()
    w1_d = nc.dram_tensor("w1", [128, kin * cfg.h1], bf, kind="ExternalInput").ap()
    wg1_d = nc.dram_tensor("wg1", [128, k1 * H2], bf, kind="ExternalInput").ap()
    wg2_d = nc.dram_tensor("wg2", [128, k2 * H2], bf, kind="ExternalInput").ap()
    wl2_d = nc.dram_tensor("wl2", [128, k2 * OUT], bf, kind="ExternalInput").ap()
    b1_d = nc.dram_tensor("b1", [128, kin], f32, kind="ExternalInput").ap()
    bg1_d = nc.dram_tensor("bg1", [1, H2], bf, kind="ExternalInput").ap()
    bg2_d = nc.dram_tensor("bg2", [1, H2], bf, kind="ExternalInput").ap()
    bl2_d = nc.dram_tensor("bl2", [1, OUT], bf, kind="ExternalInput").ap()
    idx_d = nc.dram_tensor(
        "idx", [128, meta["idxcols"]], i16, kind="ExternalInput"
    ).ap()
    lrt_d = nc.dram_tensor("lrt", [128, totch], bf, kind="ExternalInput").ap()
    wt_d = nc.dram_tensor("wt", [128, totch], bf, kind="ExternalInput").ap()
    y_d = nc.dram_tensor("y", [npad, OUT], f32, kind="ExternalOutput").ap()

    g1locA = nc.dram_tensor("g1locA", [cfg.rows_a, H2], bf).ap()
    g1locB = nc.dram_tensor("g1locB", [cfg.rows_b, H2], bf).ap()
    g2locA = nc.dram_tensor("g2locA", [cfg.rows_a, H2], bf).ap()
    g2locB = nc.dram_tensor("g2locB", [cfg.rows_b, H2], bf).ap()
    g1tabA = nc.dram_tensor("g1tabA", [cfg.tab_a, H2], bf, addr_space="Shared").ap()
    g1tabB = nc.dram_tensor("g1tabB", [cfg.tab_b, H2], bf, addr_space="Shared").ap()
    g2tabA = nc.dram_tensor("g2tabA", [cfg.tab_a, H2], bf, addr_space="Shared").ap()
    g2tabB = nc.dram_tensor("g2tabB", [cfg.tab_b, H2], bf, addr_space="Shared").ap()

    rg = [list(range(cfg.n_cores))]
    qctr = [0]  # round-robin SWDGE queue counter

    def fire_ag(src, dst):
        nc.gpsimd.collective_compute(
            "AllGather",
            mybir.AluOpType.bypass,
            replica_groups=rg,
            ins=[src[:, :]],
            outs=[dst[:, :]],
        )

    with tile.TileContext(nc) as tc:
        with ExitStack() as top:
            const = top.enter_context(tc.tile_pool(name="const", bufs=1))
            w1_s = const.tile([128, kin * cfg.h1], bf)
            nc.sync.dma_start(w1_s[:], w1_d[:, :])
            wg1_s = const.tile([128, k1 * H2], bf)
            nc.sync.dma_start(wg1_s[:], wg1_d[:, :])
            wg2_s = const.tile([128, k2 * H2], bf)
            nc.sync.dma_start(wg2_s[:], wg2_d[:, :])
            wl2_s = const.tile([128, k2 * OUT], bf)
            nc.sync.dma_start(wl2_s[:], wl2_d[:, :])
            b1_s = const.tile([128, kin], f32)
            nc.sync.dma_start(b1_s[:], b1_d[:, :])
            bg1_s = const.tile([1, H2], bf)
            nc.sync.dma_start(bg1_s[:], bg1_d[:, :])
            bg2_s = const.tile([1, H2], bf)
            nc.sync.dma_start(bg2_s[:], bg2_d[:, :])
            bl2_s = const.tile([1, OUT], bf)
            nc.sync.dma_start(bl2_s[:], bl2_d[:, :])
            idx_s = const.tile([128, meta["idxcols"]], i16)
            nc.sync.dma_start(idx_s[:], idx_d[:, :])
            lrt_s = const.tile([128, totch], bf)
            nc.sync.dma_start(lrt_s[:], lrt_d[:, :])
            wt_s = const.tile([128, totch], bf)
            nc.sync.dma_start(wt_s[:], wt_d[:, :])
            iota_i = const.tile([128, 128], i16)
            nc.gpsimd.iota(iota_i[:], pattern=[[1, 128]], base=0, channel_multiplier=0)
            iota_b = const.tile([128, 128], bf)
            nc.vector.tensor_copy(iota_b[:], iota_i[:])
            ident = const.tile([128, 128], bf)
            make_identity(nc, ident[:])
            ones_t = const.tile([1, 128], bf)
            nc.gpsimd.memset(ones_t[:], 1.0)

            # ---------------- L1 + L2a, slab-streamed; fire AG1 chunks early
            slabs = []
            b0 = 0
            while b0 < nb:
                b1e = min(b0 + cfg.slab_blocks, nb)
                slabs.append((b0, b1e))
                b0 = b1e

            with ExitStack() as pl1:
                xp = pl1.enter_context(tc.tile_pool(name="xt", bufs=2))
                hp = pl1.enter_context(tc.tile_pool(name="h1s", bufs=2))
                ps1 = pl1.enter_context(
                    tc.tile_pool(name="ps1", bufs=4, space="PSUM")
                )
                ps2 = pl1.enter_context(
                    tc.tile_pool(name="ps2", bufs=2, space="PSUM")
                )
                gp1 = pl1.enter_context(tc.tile_pool(name="g1t", bufs=3))
                for (bs, be) in slabs:
                    a = bs * 128
                    S = (be - bs) * 128
                    xs = xp.tile([128, kin, S], bf, tag="x")
                    for kt in range(kin):
                        nc.sync.dma_start(
                            xs[:, kt, :], xt_d[:, kt * npad + a : kt * npad + a + S]
                        )
                    h1s = hp.tile([128, k1, S], bf, tag="h")
                    for f1t in range(k1):
                        ps = ps1.tile([128, S], f32, tag="ps")
                        for kt in range(kin):
                            nc.tensor.matmul(
                                ps[:],
                                lhsT=w1_s[
                                    :,
                                    kt * cfg.h1 + f1t * 128 : kt * cfg.h1
                                    + f1t * 128
                                    + 128,
                                ],
                                rhs=xs[:, kt, :],
                                start=(kt == 0),
                                stop=(kt == kin - 1),
                            )
                        nc.scalar.activation(
                            h1s[:, f1t, :],
                            ps[:],
                            AF.Sigmoid,
                            bias=b1_s[:, f1t : f1t + 1],
                        )
                    for b in range(bs, be):
                        o = (b - bs) * 128
                        ps = ps2.tile([128, H2], f32, tag="ps")
                        for kt in range(k1):
                            nc.tensor.matmul(
                                ps[:],
                                lhsT=h1s[:, kt, o : o + 128],
                                rhs=wg1_s[:, kt * H2 : (kt + 1) * H2],
                                start=(kt == 0),
                                stop=(kt == k1 - 1),
                            )
                        g1t = gp1.tile([128, H2], bf, tag="g1")
                        nc.vector.tensor_copy(g1t[:], ps[:])
                        if b < SB:
                            nc.sync.dma_start(
                                g1locA[b * 128 : (b + 1) * 128, :], g1t[:]
                            )
                        else:
                            bb = b - SB
                            nc.sync.dma_start(
                                g1locB[bb * 128 : (bb + 1) * 128, :], g1t[:]
                            )
                    if be == SB:
                        fire_ag(g1locA, g1tabA)
                assert SB in [be for (_, be) in slabs], "split must be slab-aligned"
                fire_ag(g1locB, g1tabB)

            # Dependency-free PE filler: keeps the HAM activity window hot
            # across phase transitions where the PE would otherwise idle
            # waiting for an AllGather to land (idle >3.4us drops the PE
            # clock from 2.4 to 1.2 GHz for the next stretch of work).
            fillp = top.enter_context(
                tc.tile_pool(name="fill", bufs=1, space="PSUM")
            )

            def pe_filler(n_mm, tag):
                fps = fillp.tile([128, 256], f32, tag="f", name=f"f{tag}")
                for i in range(n_mm):
                    nc.tensor.matmul(
                        fps[:],
                        lhsT=w1_s[:, :128],
                        rhs=w1_s[:, :256],
                        start=True,
                        stop=True,
                    )

            # ---------------- shared spmm machinery (pools shared across
            # both layers so recycled gather tiles always hold prior gather
            # data -- required for the trailing -1 idx-pad trim)
            nch_max = max(max(r) for r in nch)
            nctot_max = max(r[0] + r[1] for r in nch)
            gp = top.enter_context(tc.tile_pool(name="gath", bufs=10))
            pp = top.enter_context(tc.tile_pool(name="pm", bufs=5))
            sp = top.enter_context(tc.tile_pool(name="psmm", bufs=4, space="PSUM"))

            def spmm_layer(ctx, tabA, tabB, brow, out_cb, tag):
                for b in range(nb):
                    tiles = {}
                    for h in (0, 1):
                        n = nch[b][h]
                        t = gp.tile([128, nch_max, H2], bf, tag="g")
                        tab = tabA if h == 0 else tabB
                        for lo in range(0, n, 15):
                            ns = min(15, n - lo)
                            o16 = off16[b][h] + lo * 8
                            nc.gpsimd.dma_gather(
                                out_ap=t[:, lo : lo + ns, :],
                                in_ap=tab[:, :],
                                idxs_ap=idx_s[:, o16 : o16 + ns * 8],
                                num_idxs=ns * 128,
                                num_idxs_reg=ns * 128,
                                elem_size=H2,
                                single_packet=False,
                                queue_num=qctr[0] % 4,
                            )
                            qctr[0] += 1
                        tiles[h] = t
                    nctot = nch[b][0] + nch[b][1]
                    j0 = offch[b][0]
                    pt = pp.tile([128, nctot_max, 128], bf, tag="p")
                    nc.vector.tensor_tensor(
                        pt[:, :nctot, :],
                        iota_b[:].unsqueeze(1).broadcast_to((128, nctot, 128)),
                        lrt_s[:, j0 : j0 + nctot]
                        .unsqueeze(2)
                        .broadcast_to((128, nctot, 128)),
                        ALU.is_equal,
                    )
                    nc.vector.tensor_tensor(
                        pt[:, :nctot, :],
                        pt[:, :nctot, :],
                        wt_s[:, j0 : j0 + nctot]
                        .unsqueeze(2)
                        .broadcast_to((128, nctot, 128)),
                        ALU.mult,
                    )
                    ps = sp.tile([128, H2], f32, tag="ps", name=f"psmm{tag}_{b}")
                    first = True
                    for h in (0, 1):
                        for j in range(nch[b][h]):
                            jl = (offch[b][h] - offch[b][0]) + j
                            nc.tensor.matmul(
                                ps[:],
                                lhsT=pt[:, jl, :],
                                rhs=tiles[h][:, j, :],
                                start=first,
                                stop=False,
                            )
                            first = False
                    nc.tensor.matmul(
                        ps[:],
                        lhsT=ones_t[:1, :],
                        rhs=brow[:1, :],
                        start=first,
                        stop=True,
                    )
                    out_cb(b, ps)

            # ---------------- spmm1 + L3a fused per block; fire AG2 chunks
            with ExitStack() as s1:
                tps3 = s1.enter_context(
                    tc.tile_pool(name="tps3", bufs=1, space="PSUM")
                )
                psp3 = s1.enter_context(
                    tc.tile_pool(name="ps3", bufs=2, space="PSUM")
                )
                tp3 = s1.enter_context(tc.tile_pool(name="l3t", bufs=3))

                def cb1(b, psum):
                    h2t = tp3.tile([128, H2], bf, tag="h2")
                    nc.scalar.activation(h2t[:], psum[:], AF.Relu)
                    h2T = tp3.tile([128, k2, 128], bf, tag="h2T")
                    for kt in range(k2):
                        ptt = tps3.tile([128, 128], bf, tag="pt")
                        nc.tensor.transpose(
                            ptt[:], h2t[:, kt * 128 : (kt + 1) * 128], ident[:]
                        )
                        nc.scalar.activation(h2T[:, kt, :], ptt[:], AF.Copy)
                    ps3 = psp3.tile([128, H2], f32, tag="ps")
                    for kt in range(k2):
                        nc.tensor.matmul(
                            ps3[:],
                            lhsT=h2T[:, kt, :],
                            rhs=wg2_s[:, kt * H2 : (kt + 1) * H2],
                            start=(kt == 0),
                            stop=(kt == k2 - 1),
                        )
                    g2t = tp3.tile([128, H2], bf, tag="g2")
                    nc.scalar.activation(g2t[:], ps3[:], AF.Copy)
                    if b < SB:
                        nc.sync.dma_start(g2locA[b * 128 : (b + 1) * 128, :], g2t[:])
                        if b == SB - 1:
                            fire_ag(g2locA, g2tabA)
                    else:
                        bb = b - SB
                        nc.sync.dma_start(
                            g2locB[bb * 128 : (bb + 1) * 128, :], g2t[:]
                        )
                        if b == nb - 1:
                            fire_ag(g2locB, g2tabB)

                pe_filler(150, "a")
                spmm_layer(s1, g1tabA, g1tabB, bg1_s, cb1, "a")

            # ---------------- spmm2 + L4 fused per block
            with ExitStack() as s2:
                tps4 = s2.enter_context(
                    tc.tile_pool(name="tps4", bufs=1, space="PSUM")
                )
                psp4 = s2.enter_context(
                    tc.tile_pool(name="ps4", bufs=2, space="PSUM")
                )
                tp4 = s2.enter_context(tc.tile_pool(name="l4t", bufs=3))

                def cb2(b, psum):
                    h3t = tp4.tile([128, H2], bf, tag="h3")
                    nc.scalar.activation(h3t[:], psum[:], AF.Relu)
                    h3T = tp4.tile([128, k2, 128], bf, tag="h3T")
                    for kt in range(k2):
                        ptt = tps4.tile([128, 128], bf, tag="pt")
                        nc.tensor.transpose(
                            ptt[:], h3t[:, kt * 128 : (kt + 1) * 128], ident[:]
                        )
                        nc.scalar.activation(h3T[:, kt, :], ptt[:], AF.Copy)
                    ps4 = psp4.tile([128, OUT], f32, tag="ps")
                    for kt in range(k2):
                        nc.tensor.matmul(
                            ps4[:],
                            lhsT=h3T[:, kt, :],
                            rhs=wl2_s[:, kt * OUT : (kt + 1) * OUT],
                            start=(kt == 0),
                            stop=False,
                        )
                    nc.tensor.matmul(
                        ps4[:],
                        lhsT=ones_t[:1, :],
                        rhs=bl2_s[:1, :],
                        start=False,
                        stop=True,
                    )
                    yt = tp4.tile([128, OUT], f32, tag="y")
                    nc.scalar.activation(yt[:], ps4[:], AF.Copy)
                    nc.sync.dma_start(y_d[b * 128 : (b + 1) * 128, :], yt[:])

                pe_filler(100, "b")
                spmm_layer(s2, g2tabA, g2tabB, bg2_s, cb2, "b")

    nc.compile()
    return nc


# ---------------------------------------------------------------- driver

_CACHE = {}


def run(inputs, cfg: Cfg = FULL, trace=False, tmpdir=None):
    meta, in_maps = prep_inputs(cfg, inputs)
    key = (cfg, meta["totch"], meta["idxcols"])
    if key not in _CACHE:
        _CACHE[key] = build(cfg, meta)
    nc = _CACHE[key]
    res = run_bass_kernel_spmd(
        nc,
        in_maps,
        core_ids=list(range(cfg.n_cores)),
        trace=trace,
        tmpdir=tmpdir,
    )
    npc = cfg.nodes_per_core
    out = np.empty((cfg.n_nodes, cfg.out_dim), np.float32)
    for c in range(cfg.n_cores):
        lo = c * npc
        hi = min((c + 1) * npc, cfg.n_nodes)
        out[lo:hi] = res.results[c]["y"][: hi - lo]
    return out, res


def kernel(**inputs) -> np.ndarray:
    out, _ = run(inputs, FULL, trace=False)
    return out


# revision 32
# speedup vs baseline: 1.2468x; 1.2036x over previous
"""GNN message-passing kernel for 8 Trainium2 NeuronCores (Bass/Tile).

Takes FULL inputs, shards nodes across 8 cores internally, runs the
4-layer GNN (dense -> spmm -> spmm -> dense) with two bf16 AllGathers
of the hidden node table (each split in two chunks, fired as soon as
the producing blocks finish), and PE-matmul-based weighted segment
sums.  The one-hot selector matrices are built ON-CHIP on the vector
engine from per-edge (local-row, weight) pairs; the row gathers are
spread round-robin over 4 SWDGE queues so descriptor generation runs
on all four GpSimd Q7 core-pairs in parallel.
"""

import math
from contextlib import ExitStack
from dataclasses import dataclass

import ml_dtypes
import numpy as np

import concourse.bass as bass
import concourse.mybir as mybir
import concourse.tile as tile
from concourse import bacc
from concourse.bass_utils import run_bass_kernel_spmd
from concourse.masks import make_identity

BF16 = ml_dtypes.bfloat16
AF = mybir.ActivationFunctionType
ALU = mybir.AluOpType


@dataclass(frozen=True)
class Cfg:
    n_nodes: int = 50000
    n_edges: int = 800000
    in_dim: int = 512
    h1: int = 512
    h2: int = 256
    out_dim: int = 128
    n_cores: int = 8
    split_block: int = 28  # blocks [0, 28) -> half A, [28, 49) -> half B
    slab_blocks: int = 4   # L1/L2a node-slab width in 128-blocks

    @property
    def nodes_per_core(self):
        return self.n_nodes // self.n_cores  # 6250

    @property
    def npad(self):
        return math.ceil(self.nodes_per_core / 128) * 128  # 6272

    @property
    def nblocks(self):
        return self.npad // 128  # 49

    @property
    def rows_a(self):
        return self.split_block * 128  # 3584

    @property
    def rows_b(self):
        return self.npad - self.rows_a  # 2688

    @property
    def tab_a(self):
        return self.rows_a * self.n_cores  # 28672

    @property
    def tab_b(self):
        return self.rows_b * self.n_cores  # 21504


FULL = Cfg()


# ---------------------------------------------------------------- host prep


def edge_structure(cfg: Cfg, edge_row, edge_col, edge_weight):
    """Bucket edges per (core, row-block, table-half); uniform chunk counts.

    Returns (meta, per_core):
      meta['nch'][b][h]   chunks for block b, half h (same on all cores)
      meta['off16'][b][h] idx-tile int16-column offset of that bucket
      meta['offch'][b][h] chunk offset (for the lrw tile)
      meta['totch']       total chunks
      meta['idxcols']     int16 columns of the idx tensor
    per_core[c] = dict(idx=[128, idxcols] int16, lrw=[128, totch*2] bf16)
    """
    nc_, npad, nb = cfg.n_cores, cfg.npad, cfg.nblocks
    npc = cfg.nodes_per_core
    rows_a = cfg.rows_a
    assert cfg.tab_a <= 32767 and cfg.tab_b <= 32767

    core_of = edge_row // npc
    lr_all = edge_row - core_of * npc          # local dest row on owner core
    cc_of = edge_col // npc                    # core owning the source col
    cl_all = edge_col - cc_of * npc            # local source row
    half_all = (cl_all >= rows_a).astype(np.int64)
    # index within the half table
    tidx_all = np.where(
        half_all == 0,
        cc_of * rows_a + cl_all,
        cc_of * cfg.rows_b + (cl_all - rows_a),
    )

    counts = np.zeros((nc_, nb, 2), np.int64)
    per = {}
    for c in range(nc_):
        m = core_of == c
        lr, ti, hf, w = lr_all[m], tidx_all[m], half_all[m], edge_weight[m]
        blk = lr // 128
        order = np.lexsort((ti, hf, blk))  # sort by (block, half, table idx)
        per[c] = (lr[order], ti[order], hf[order], w[order], blk[order])
        np.add.at(counts[c], (blk, hf), 1)

    chunks_bh = np.ceil(counts / 128.0).astype(np.int64).max(axis=0)  # [nb,2]
    chunks_bh = np.maximum(chunks_bh, 1)

    nch = [[int(chunks_bh[b, h]) for h in (0, 1)] for b in range(nb)]
    off16 = [[0, 0] for _ in range(nb)]
    offch = [[0, 0] for _ in range(nb)]
    tot16 = 0
    totch = 0
    for b in range(nb):
        for h in (0, 1):
            off16[b][h] = tot16
            offch[b][h] = totch
            tot16 += nch[b][h] * 8  # 128 idx per chunk -> 8 int16 cols
            totch += nch[b][h]

    meta = dict(nch=nch, off16=off16, offch=offch, totch=totch, idxcols=tot16)

    per_core = []
    for c in range(nc_):
        lr, ti, hf, w, blk = per[c]
        idx_flat = np.zeros(tot16 * 16, np.int16)
        lr_tab = np.zeros((128, totch), BF16)
        w_tab = np.zeros((128, totch), BF16)
        bucket_no = 0
        p = 0  # cursor into sorted edge stream
        for b in range(nb):
            for h in (0, 1):
                q = p
                while q < len(blk) and blk[q] == b and hf[q] == h:
                    q += 1
                e_ti, e_lr, e_w = ti[p:q], lr[p:q], w[p:q]
                p = q
                n = len(e_ti)
                nslots = nch[b][h] * 128
                pad = nslots - n
                # pad slots carry w=0, so any valid index works; reuse the
                # bucket's last real index so pad fetches stay on the same
                # DRAM page as the preceding real fetch.
                pad_idx = int(e_ti[-1]) if n > 0 else 0
                bucket_no += 1
                ti_pad = np.concatenate(
                    [e_ti, np.full(pad, pad_idx, np.int64)]
                )
                i_in = np.arange(nslots)
                base16 = off16[b][h]
                idx_flat[(base16 + i_in // 16) * 16 + (i_in % 16)] = ti_pad.astype(
                    np.int16
                )
                if n > 0:
                    j0 = offch[b][h]
                    i_e = np.arange(n)
                    jj = j0 + i_e // 128
                    ss = i_e % 128
                    lr_tab[ss, jj] = (e_lr - b * 128).astype(BF16)
                    w_tab[ss, jj] = e_w.astype(BF16)
        idx_mat = idx_flat.reshape(tot16, 16).T  # [16, idxcols]
        idx_mat = np.tile(idx_mat, (8, 1))       # replicate to 128 partitions
        per_core.append(
            dict(
                idx=np.ascontiguousarray(idx_mat),
                lrt=np.ascontiguousarray(lr_tab),
                wt=np.ascontiguousarray(w_tab),
            )
        )
    return meta, per_core


def prep_inputs(cfg: Cfg, inputs):
    f = inputs["features"].astype(np.float32)
    meta, per_edge = edge_structure(
        cfg,
        inputs["edge_row"].astype(np.int64),
        inputs["edge_col"].astype(np.int64),
        inputs["edge_weight"].astype(np.float32),
    )
    kin = cfg.in_dim // 128
    k1 = cfg.h1 // 128
    k2 = cfg.h2 // 128

    def wlayout(w, kt):
        K, M = w.shape
        return (
            w.reshape(kt, 128, M).transpose(1, 0, 2).reshape(128, kt * M)
        ).astype(BF16)

    w1 = wlayout(inputs["W_lin1"].astype(np.float32), kin)
    wg1 = wlayout(inputs["W_g1"].astype(np.float32), k1)
    wg2 = wlayout(inputs["W_g2"].astype(np.float32), k2)
    wl2 = wlayout(inputs["W_lin2"].astype(np.float32), k2)
    b1 = inputs["b_lin1"].astype(np.float32).reshape(kin, 128).T.copy()
    bg1 = inputs["b_g1"].astype(BF16).reshape(1, cfg.h2)
    bg2 = inputs["b_g2"].astype(BF16).reshape(1, cfg.h2)
    bl2 = inputs["b_lin2"].astype(BF16).reshape(1, cfg.out_dim)

    npc, npad = cfg.nodes_per_core, cfg.npad
    in_maps = []
    for c in range(cfg.n_cores):
        lo = c * npc
        hi = min((c + 1) * npc, cfg.n_nodes)
        xc = np.zeros((npad, cfg.in_dim), np.float32)
        xc[: hi - lo] = f[lo:hi]
        xt = (
            xc.T.reshape(kin, 128, npad)
            .transpose(1, 0, 2)
            .reshape(128, kin * npad)
        ).astype(BF16)
        in_maps.append(
            {
                "xt": np.ascontiguousarray(xt),
                "w1": w1,
                "wg1": wg1,
                "wg2": wg2,
                "wl2": wl2,
                "b1": b1,
                "bg1": bg1,
                "bg2": bg2,
                "bl2": bl2,
                "idx": per_edge[c]["idx"],
                "lrt": per_edge[c]["lrt"],
                "wt": per_edge[c]["wt"],
            }
        )
    return meta, in_maps


# ---------------------------------------------------------------- kernel IR


def build(cfg: Cfg, meta):
    nc = bacc.Bacc(
        "TRN2",
        target_bir_lowering=False,
        debug=False,
        num_devices=cfg.n_cores,
        num_swdge_queues=4,
    )
    bf = mybir.dt.bfloat16
    f32 = mybir.dt.float32
    i16 = mybir.dt.int16
    kin = cfg.in_dim // 128
    k1 = cfg.h1 // 128
    k2 = cfg.h2 // 128
    npad, nb, H2, OUT = cfg.npad, cfg.nblocks, cfg.h2, cfg.out_dim
    SB = cfg.split_block
    totch = meta["totch"]
    nch = meta["nch"]
    off16 = meta["off16"]
    offch = meta["offch"]

    xt_d = nc.dram_tensor("xt", [128, kin * npad], bf, kind="ExternalInput").ap()
    w1_d = nc.dram_tensor("w1", [128, kin * cfg.h1], bf, kind="ExternalInput").ap()
    wg1_d = nc.dram_tensor("wg1", [128, k1 * H2], bf, kind="ExternalInput").ap()
    wg2_d = nc.dram_tensor("wg2", [128, k2 * H2], bf, kind="ExternalInput").ap()
    wl2_d = nc.dram_tensor("wl2", [128, k2 * OUT], bf, kind="ExternalInput").ap()
    b1_d = nc.dram_tensor("b1", [128, kin], f32, kind="ExternalInput").ap()
    bg1_d = nc.dram_tensor("bg1", [1, H2], bf, kind="ExternalInput").ap()
    bg2_d = nc.dram_tensor("bg2", [1, H2], bf, kind="ExternalInput").ap()
    bl2_d = nc.dram_tensor("bl2", [1, OUT], bf, kind="ExternalInput").ap()
    idx_d = nc.dram_tensor(
        "idx", [128, meta["idxcols"]], i16, kind="ExternalInput"
    ).ap()
    lrt_d = nc.dram_tensor("lrt", [128, totch], bf, kind="ExternalInput").ap()
    wt_d = nc.dram_tensor("wt", [128, totch], bf, kind="ExternalInput").ap()
    y_d = nc.dram_tensor("y", [npad, OUT], f32, kind="ExternalOutput").ap()

    g1locA = nc.dram_tensor("g1locA", [cfg.rows_a, H2], bf).ap()
    g1locB = nc.dram_tensor("g1locB", [cfg.rows_b, H2], bf).ap()
    g2locA = nc.dram_tensor("g2locA", [cfg.rows_a, H2], bf).ap()
    g2locB = nc.dram_tensor("g2locB", [cfg.rows_b, H2], bf).ap()
    g1tabA = nc.dram_tensor("g1tabA", [cfg.tab_a, H2], bf, addr_space="Shared").ap()
    g1tabB = nc.dram_tensor("g1tabB", [cfg.tab_b, H2], bf, addr_space="Shared").ap()
    g2tabA = nc.dram_tensor("g2tabA", [cfg.tab_a, H2], bf, addr_space="Shared").ap()
    g2tabB = nc.dram_tensor("g2tabB", [cfg.tab_b, H2], bf, addr_space="Shared").ap()

    rg = [list(range(cfg.n_cores))]
    qctr = [0]  # round-robin SWDGE queue counter

    def fire_ag(src, dst):
        nc.gpsimd.collective_compute(
            "AllGather",
            mybir.AluOpType.bypass,
            replica_groups=rg,
            ins=[src[:, :]],
            outs=[dst[:, :]],
        )

    with tile.TileContext(nc) as tc:
        with ExitStack() as top:
            const = top.enter_context(tc.tile_pool(name="const", bufs=1))
            w1_s = const.tile([128, kin * cfg.h1], bf)
            nc.sync.dma_start(w1_s[:], w1_d[:, :])
            wg1_s = const.tile([128, k1 * H2], bf)
            nc.sync.dma_start(wg1_s[:], wg1_d[:, :])
            wg2_s = const.tile([128, k2 * H2], bf)
            nc.sync.dma_start(wg2_s[:], wg2_d[:, :])
            wl2_s = const.tile([128, k2 * OUT], bf)
            nc.sync.dma_start(wl2_s[:], wl2_d[:, :])
            b1_s = const.tile([128, kin], f32)
            nc.sync.dma_start(b1_s[:], b1_d[:, :])
            bg1_s = const.tile([1, H2], bf)
            nc.sync.dma_start(bg1_s[:], bg1_d[:, :])
            bg2_s = const.tile([1, H2], bf)
            nc.sync.dma_start(bg2_s[:], bg2_d[:, :])
            bl2_s = const.tile([1, OUT], bf)
            nc.sync.dma_start(bl2_s[:], bl2_d[:, :])
            idx_s = const.tile([128, meta["idxcols"]], i16)
            nc.sync.dma_start(idx_s[:], idx_d[:, :])
            lrt_s = const.tile([128, totch], bf)
            nc.sync.dma_start(lrt_s[:], lrt_d[:, :])
            wt_s = const.tile([128, totch], bf)
            nc.sync.dma_start(wt_s[:], wt_d[:, :])
            iota_i = const.tile([128, 128], i16)
            nc.gpsimd.iota(iota_i[:], pattern=[[1, 128]], base=0, channel_multiplier=0)
            iota_b = const.tile([128, 128], bf)
            nc.vector.tensor_copy(iota_b[:], iota_i[:])
            ident = const.tile([128, 128], bf)
            make_identity(nc, ident[:])
            ones_t = const.tile([1, 128], bf)
            nc.gpsimd.memset(ones_t[:], 1.0)

            # ---------------- L1 + L2a, slab-streamed; fire AG1 chunks early
            slabs = []
            b0 = 0
            while b0 < nb:
                b1e = min(b0 + cfg.slab_blocks, nb)
                slabs.append((b0, b1e))
                b0 = b1e

            with ExitStack() as pl1:
                xp = pl1.enter_context(tc.tile_pool(name="xt", bufs=2))
                hp = pl1.enter_context(tc.tile_pool(name="h1s", bufs=2))
                ps1 = pl1.enter_context(
                    tc.tile_pool(name="ps1", bufs=4, space="PSUM")
                )
                ps2 = pl1.enter_context(
                    tc.tile_pool(name="ps2", bufs=2, space="PSUM")
                )
                gp1 = pl1.enter_context(tc.tile_pool(name="g1t", bufs=3))
                for (bs, be) in slabs:
                    a = bs * 128
                    S = (be - bs) * 128
                    xs = xp.tile([128, kin, S], bf, tag="x")
                    for kt in range(kin):
                        nc.sync.dma_start(
                            xs[:, kt, :], xt_d[:, kt * npad + a : kt * npad + a + S]
                        )
                    h1s = hp.tile([128, k1, S], bf, tag="h")
                    for f1t in range(k1):
                        ps = ps1.tile([128, S], f32, tag="ps")
                        for kt in range(kin):
                            nc.tensor.matmul(
                                ps[:],
                                lhsT=w1_s[
                                    :,
                                    kt * cfg.h1 + f1t * 128 : kt * cfg.h1
                                    + f1t * 128
                                    + 128,
                                ],
                                rhs=xs[:, kt, :],
                                start=(kt == 0),
                                stop=(kt == kin - 1),
                            )
                        nc.scalar.activation(
                            h1s[:, f1t, :],
                            ps[:],
                            AF.Sigmoid,
                            bias=b1_s[:, f1t : f1t + 1],
                        )
                    for b in range(bs, be):
                        o = (b - bs) * 128
                        ps = ps2.tile([128, H2], f32, tag="ps")
                        for kt in range(k1):
                            nc.tensor.matmul(
                                ps[:],
                                lhsT=h1s[:, kt, o : o + 128],
                                rhs=wg1_s[:, kt * H2 : (kt + 1) * H2],
                                start=(kt == 0),
                                stop=(kt == k1 - 1),
                            )
                        g1t = gp1.tile([128, H2], bf, tag="g1")
                        nc.vector.tensor_copy(g1t[:], ps[:])
                        if b < SB:
                            nc.sync.dma_start(
                                g1locA[b * 128 : (b + 1) * 128, :], g1t[:]
                            )
                        else:
                            bb = b - SB
                            nc.sync.dma_start(
                                g1locB[bb * 128 : (bb + 1) * 128, :], g1t[:]
                            )
                    if be == SB:
                        fire_ag(g1locA, g1tabA)
                assert SB in [be for (_, be) in slabs], "split must be slab-aligned"
                fire_ag(g1locB, g1tabB)

            # Dependency-free PE filler: keeps the HAM activity window hot
            # across phase transitions where the PE would otherwise idle
            # waiting for an AllGather to land (idle >3.4us drops the PE
            # clock from 2.4 to 1.2 GHz for the next stretch of work).
            fillp = top.enter_context(
                tc.tile_pool(name="fill", bufs=1, space="PSUM")
            )

            def pe_filler(n_mm, tag):
                fps = fillp.tile([128, 256], f32, tag="f", name=f"f{tag}")
                for i in range(n_mm):
                    nc.tensor.matmul(
                        fps[:],
                        lhsT=w1_s[:, :128],
                        rhs=w1_s[:, :256],
                        start=True,
                        stop=True,
                    )

            # ---------------- shared spmm machinery (pools shared across
            # both layers so recycled gather tiles always hold prior gather
            # data -- required for the trailing -1 idx-pad trim)
            nch_max = max(max(r) for r in nch)
            nctot_max = max(r[0] + r[1] for r in nch)
            gp = top.enter_context(tc.tile_pool(name="gath", bufs=10))
            pp = top.enter_context(tc.tile_pool(name="pm", bufs=5))
            sp = top.enter_context(tc.tile_pool(name="psmm", bufs=4, space="PSUM"))

            def spmm_layer(ctx, tabA, tabB, brow, out_cb, tag):
                for b in range(nb):
                    tiles = {}
                    for h in (0, 1):
                        n = nch[b][h]
                        t = gp.tile([128, nch_max, H2], bf, tag="g")
                        tab = tabA if h == 0 else tabB
                        for lo in range(0, n, 15):
                            ns = min(15, n - lo)
                            o16 = off16[b][h] + lo * 8
                            nc.gpsimd.dma_gather(
                                out_ap=t[:, lo : lo + ns, :],
                                in_ap=tab[:, :],
                                idxs_ap=idx_s[:, o16 : o16 + ns * 8],
                                num_idxs=ns * 128,
                                num_idxs_reg=ns * 128,
                                elem_size=H2,
                                single_packet=False,
                                queue_num=qctr[0] % 4,
                            )
                            qctr[0] += 1
                        tiles[h] = t
                    nctot = nch[b][0] + nch[b][1]
                    j0 = offch[b][0]
                    pt = pp.tile([128, nctot_max, 128], bf, tag="p")
                    nc.vector.tensor_tensor(
                        pt[:, :nctot, :],
                        iota_b[:].unsqueeze(1).broadcast_to((128, nctot, 128)),
                        lrt_s[:, j0 : j0 + nctot]
                        .unsqueeze(2)
                        .broadcast_to((128, nctot, 128)),
                        ALU.is_equal,
                    )
                    nc.vector.tensor_tensor(
                        pt[:, :nctot, :],
                        pt[:, :nctot, :],
                        wt_s[:, j0 : j0 + nctot]
                        .unsqueeze(2)
                        .broadcast_to((128, nctot, 128)),
                        ALU.mult,
                    )
                    ps = sp.tile([128, H2], f32, tag="ps", name=f"psmm{tag}_{b}")
                    first = True
                    for h in (0, 1):
                        for j in range(nch[b][h]):
                            jl = (offch[b][h] - offch[b][0]) + j
                            nc.tensor.matmul(
                                ps[:],
                                lhsT=pt[:, jl, :],
                                rhs=tiles[h][:, j, :],
                                start=first,
                                stop=False,
                            )
                            first = False
                    nc.tensor.matmul(
                        ps[:],
                        lhsT=ones_t[:1, :],
                        rhs=brow[:1, :],
                        start=first,
                        stop=True,
                    )
                    out_cb(b, ps)

            # ---------------- spmm1 + L3a fused per block; fire AG2 chunks
            with ExitStack() as s1:
                tps3 = s1.enter_context(
                    tc.tile_pool(name="tps3", bufs=1, space="PSUM")
                )
                psp3 = s1.enter_context(
                    tc.tile_pool(name="ps3", bufs=2, space="PSUM")
                )
                tp3 = s1.enter_context(tc.tile_pool(name="l3t", bufs=3))

                def cb1(b, psum):
                    h2t = tp3.tile([128, H2], bf, tag="h2")
                    nc.scalar.activation(h2t[:], psum[:], AF.Relu)
                    h2T = tp3.tile([128, k2, 128], bf, tag="h2T")
                    for kt in range(k2):
                        ptt = tps3.tile([128, 128], bf, tag="pt")
                        nc.tensor.transpose(
                            ptt[:], h2t[:, kt * 128 : (kt + 1) * 128], ident[:]
                        )
                        nc.scalar.activation(h2T[:, kt, :], ptt[:], AF.Copy)
                    ps3 = psp3.tile([128, H2], f32, tag="ps")
                    for kt in range(k2):
                        nc.tensor.matmul(
                            ps3[:],
                            lhsT=h2T[:, kt, :],
                            rhs=wg2_s[:, kt * H2 : (kt + 1) * H2],
                            start=(kt == 0),
                            stop=(kt == k2 - 1),
                        )
                    g2t = tp3.tile([128, H2], bf, tag="g2")
                    nc.scalar.activation(g2t[:], ps3[:], AF.Copy)
                    if b < SB:
                        nc.sync.dma_start(g2locA[b * 128 : (b + 1) * 128, :], g2t[:])
                        if b == SB - 1:
                            fire_ag(g2locA, g2tabA)
                    else:
                        bb = b - SB
                        nc.sync.dma_start(
                            g2locB[bb * 128 : (bb + 1) * 128, :], g2t[:]
                        )
                        if b == nb - 1:
                            fire_ag(g2locB, g2tabB)

                pe_filler(150, "a")
                spmm_layer(s1, g1tabA, g1tabB, bg1_s, cb1, "a")

            # ---------------- spmm2 + L4 fused per block
            with ExitStack() as s2:
                tps4 = s2.enter_context(
                    tc.tile_pool(name="tps4", bufs=1, space="PSUM")
                )
                psp4 = s2.enter_context(
                    tc.tile_pool(name="ps4", bufs=2, space="PSUM")
                )
                tp4 = s2.enter_context(tc.tile_pool(name="l4t", bufs=3))

                def cb2(b, psum):
                    h3t = tp4.tile([128, H2], bf, tag="h3")
                    nc.scalar.activation(h3t[:], psum[:], AF.Relu)
                    h3T = tp4.tile([128, k2, 128], bf, tag="h3T")
                    for kt in range(k2):
                        ptt = tps4.tile([128, 128], bf, tag="pt")
                        nc.tensor.transpose(
                            ptt[:], h3t[:, kt * 128 : (kt + 1) * 128], ident[:]
                        )
                        nc.scalar.activation(h3T[:, kt, :], ptt[:], AF.Copy)
                    ps4 = psp4.tile([128, OUT], f32, tag="ps")
                    for kt in range(k2):
                        nc.tensor.matmul(
                            ps4[:],
                            lhsT=h3T[:, kt, :],
                            rhs=wl2_s[:, kt * OUT : (kt + 1) * OUT],
                            start=(kt == 0),
                            stop=False,
                        )
                    nc.tensor.matmul(
                        ps4[:],
                        lhsT=ones_t[:1, :],
                        rhs=bl2_s[:1, :],
                        start=False,
                        stop=True,
                    )
                    yt = tp4.tile([128, OUT], f32, tag="y")
                    nc.scalar.activation(yt[:], ps4[:], AF.Copy)
                    nc.sync.dma_start(y_d[b * 128 : (b + 1) * 128, :], yt[:])

                pe_filler(100, "b")
                spmm_layer(s2, g2tabA, g2tabB, bg2_s, cb2, "b")

    nc.compile()
    return nc


# ---------------------------------------------------------------- driver

_CACHE = {}


def run(inputs, cfg: Cfg = FULL, trace=False, tmpdir=None):
    meta, in_maps = prep_inputs(cfg, inputs)
    key = (cfg, meta["totch"], meta["idxcols"])
    if key not in _CACHE:
        _CACHE[key] = build(cfg, meta)
    nc = _CACHE[key]
    res = run_bass_kernel_spmd(
        nc,
        in_maps,
        core_ids=list(range(cfg.n_cores)),
        trace=trace,
        tmpdir=tmpdir,
    )
    npc = cfg.nodes_per_core
    out = np.empty((cfg.n_nodes, cfg.out_dim), np.float32)
    for c in range(cfg.n_cores):
        lo = c * npc
        hi = min((c + 1) * npc, cfg.n_nodes)
        out[lo:hi] = res.results[c]["y"][: hi - lo]
    return out, res


def kernel(**inputs) -> np.ndarray:
    out, _ = run(inputs, FULL, trace=False)
    return out
